# revision 1
# baseline (speedup 1.0000x reference)
"""Cross-attention (B=4, N=2048, C=768, H=12, HD=64) on 8 TRN2 NeuronCores.

Sharding: core = (batch, head_group) with 4 batches x 2 groups of 6 heads
(data parallel over batch, tensor parallel over heads).  Each core computes
its group's Q/K/V projections, per-head-dim LayerNorm, attention, and a
partial output projection; the host sums the two group partials per batch
and adds the bias.

Device-side layout notes:
 - Activations are fed pre-transposed (c on partitions) so every matmul
   contracts over the partition dim without any on-device transposes.
 - q~ / k~ live as [384, 2048] (head-dim on partitions), so attention
   scores are computed transposed: S^T[k_tok, q_tok].  Softmax exp needs
   no row-max (LN bounds |S| < ~10), masked q rows are folded into the
   LN scale (rs *= mask) making their score columns exactly 0 -> uniform
   softmax, matching the reference's -1e9 row-fill semantics.
 - The softmax denominator comes free from a ones-column appended to v
   (PV matmul lhsT is [128, 65]; row 64 accumulates sum_j E[j, i]).
 - All matmuls run as float32r (full PE rate at moving dim >= 256,
   ~1e-4 relative error).  Producers write through f32r-bitcast APs to
   satisfy the compiler's "rounded to FP32r" rule.
"""

import numpy as np

import concourse.bass as bass
import concourse.mybir as mybir
from concourse import tile
from concourse import bass_utils
from concourse.tile_scheduler import N_PROCS
from concourse.vector_clock import ScopedClock, VectorClock

F32 = mybir.dt.float32
F32R = mybir.dt.float32r
AF = mybir.ActivationFunctionType
OP = mybir.AluOpType

B, N, C, H, HD = 4, 2048, 768, 12, 64
G = 2                 # head groups (tensor parallel)
HPG = H // G          # 6 heads per group
CL = HPG * HD         # 384 local channels
P = 128
CH = 512              # token chunk
NCH = N // CH         # 4
NT = CL // P          # 3 output tiles per group
CT = C // P           # 6 contraction tiles
TT = N // P           # 16 token tiles
KT_GRP = 2            # k-tiles per exp group ([128, 1024] S^T psum)
EPS = 1e-5
SCALE = HD ** -0.5
NCORES = 8

_nop_ctr = [0]


class _FixedTileContext(tile.TileContext):
    """Workaround for a walrus build that allows at most ONE sync-wait per
    instruction: split multi-wait instructions into single-wait NoOps on the
    same engine, and emit the kernel-tail drain's waits as a nop chain."""

    def _split_multiwait(self, insts):
        out = []
        for inst in insts:
            si = getattr(inst, "sync_info", None)
            waits = list(si.on_wait) if si is not None and si.on_wait else []
            if len(waits) > 1:
                eng = inst.engine
                for w in waits[:-1]:
                    _nop_ctr[0] += 1
                    nop = mybir.InstNoOp(
                        name=f"I-waitsplit-{_nop_ctr[0]}", ins=[], outs=[]
                    )
                    nop.engine = eng
                    nop.sync_info = mybir.SyncInfo(on_wait=[w], on_update=[])
                    self.nc.register_instruction(nop)
                    out.append(nop)
                inst.sync_info = mybir.SyncInfo(
                    on_wait=[waits[-1]], on_update=list(si.on_update)
                )
            out.append(inst)
        return out

    def _lower_ordered_insts(self, ordered):
        ordered = {bb: self._split_multiwait(ins) for bb, ins in ordered.items()}
        super()._lower_ordered_insts(ordered)

    def _drain_and_barrier(self, tick_clock, wait_clock):
        gc = tick_clock.global_clock
        vals = [gc[p] for p in range(N_PROCS)]
        for p in [q for q, v in enumerate(vals) if v > 0]:
            partial = VectorClock(
                [vals[q] if q == p else 0 for q in range(N_PROCS)]
            )
            nop = self.nc.sync.nop(nofuse=True, hint="tail_drain_wait")
            wait_clock.add_sem_waits(nop.ins, ScopedClock({None: partial}))
        self.nc.sync.drain()
        self.nc.all_engine_barrier()
        assert self.sems is not None
        popped = self.nc._tile_sem_poison_stack.pop()
        assert popped is self._sem_poison
        self.nc.clear_and_free_semaphores(list(self.sems.allocated().values()))
        self.nc.all_engine_barrier()


def _mm(nc, out, lhsT, rhs, start, stop):
    nc.tensor.matmul(
        out, lhsT, rhs, start=start, stop=stop, skip_group_check=True
    )


def _body(tc, aps):
    nc = tc.nc
    qxT, kvxT, wq, wk, wv, wp, msk, colsel, bcast, ones1, vones, outT = aps

    cpool = tc.alloc_tile_pool(name="consts", bufs=1)
    bpool = tc.alloc_tile_pool(name="big", bufs=1)

    colsel_sb = cpool.tile([P, NT, HPG], F32R, name="colsel", tag="colsel")
    nc.sync.dma_start(colsel_sb[:], colsel[:])
    bcast_sb = cpool.tile([HPG, NT, P], F32R, name="bcast", tag="bcast")
    nc.sync.dma_start(bcast_sb[:], bcast[:])
    ones4_sb = cpool.tile([65, HD], F32R, name="ones4", tag="ones4")
    nc.sync.dma_start(ones4_sb[:], ones1[:])
    msk_sb = cpool.tile([HPG, N], F32, name="msk", tag="msk")
    nc.sync.dma_start(msk_sb[:], msk[:])
    eps_sb = cpool.tile([HPG, 1], F32, name="eps", tag="eps")
    nc.vector.memset(eps_sb[:], EPS)

    q_sb = [bpool.tile([P, N], F32, name=f"q{t}", tag=f"q{t}") for t in range(NT)]
    k_sb = [bpool.tile([P, N], F32, name=f"k{t}", tag=f"k{t}") for t in range(NT)]
    v_sb = bpool.tile([P, TT, HPG, HD + 1], F32, name="v", tag="v")
    den_all = bpool.tile([65, HPG * CH], F32, name="den", tag="den")

    # ---------------- phase 1: projections + layernorm ----------------
    ps_t = tc.alloc_tile_pool(name="ps1", bufs=8, space="PSUM")
    w_pool = tc.alloc_tile_pool(name="wts", bufs=1)
    xq_pool = tc.alloc_tile_pool(name="xq", bufs=3)
    xkv_pool = tc.alloc_tile_pool(name="xkv", bufs=7)
    sq_pool = tc.alloc_tile_pool(name="sq", bufs=3)
    st_pool = tc.alloc_tile_pool(name="st", bufs=2)
    if True:
        wq_sb = w_pool.tile([P, CT, CL], F32R, name="wq", tag="wq")
        nc.sync.dma_start(wq_sb[:], wq.rearrange("(ct p) m -> p ct m", p=P))
        wk_sb = w_pool.tile([P, CT, CL], F32R, name="wk", tag="wk")
        wv_sb = w_pool.tile([P, CT, CL], F32R, name="wv", tag="wv")

        def ln_chunk(xT, w_sb, dst, masked, c):
            if True:
                cs = slice(c * CH, (c + 1) * CH)
                pp = [ps_t.tile([P, CH], F32, name="pt", tag="pt") for _ in range(NT)]
                xts = []
                pool = xq_pool if masked else xkv_pool
                xtag = "xq" if masked else "xkv"
                for ct in range(CT):
                    xt = pool.tile([P, CH], F32R, name=xtag, tag=xtag)
                    nc.sync.dma_start(xt[:], xT[ct * P:(ct + 1) * P, cs])
                    xts.append(xt)
                    for t in range(NT):
                        _mm(nc, pp[t][:], w_sb[:, ct, t * P:(t + 1) * P],
                            xt[:], ct == 0, ct == CT - 1)
                sqs = []
                for t in range(NT):
                    nc.vector.tensor_copy(dst[t][:, cs].bitcast(F32R), pp[t][:])
                    sq_t = sq_pool.tile([P, CH], F32, name="sq", tag="sq")
                    nc.scalar.activation(sq_t[:].bitcast(F32R), pp[t][:], AF.Square)
                    sqs.append(sq_t)
                mu_ps = ps_t.tile([HPG, CH], F32, name="pt", tag="pt")
                for t in range(NT):
                    _mm(nc, mu_ps[:], colsel_sb[:, t, :],
                        dst[t][:, cs].bitcast(F32R), t == 0, t == NT - 1)
                ms_ps = ps_t.tile([HPG, CH], F32, name="pt", tag="pt")
                for t in range(NT):
                    _mm(nc, ms_ps[:], colsel_sb[:, t, :],
                        sqs[t][:].bitcast(F32R), t == 0, t == NT - 1)
                st = st_pool.tile([HPG, 4 * CH], F32, name="st", tag="st")
                work = st[:, 0:CH]
                rs = st[:, CH:2 * CH]
                murs = st[:, 2 * CH:3 * CH]
                mu_sb = st[:, 3 * CH:4 * CH]
                nc.vector.tensor_copy(mu_sb.bitcast(F32R), mu_ps[:])
                # var = E[x^2] - mu^2
                nc.vector.scalar_tensor_tensor(
                    work.bitcast(F32R), mu_sb, 1.0, mu_sb, OP.mult, OP.mult)
                nc.vector.tensor_tensor(
                    work.bitcast(F32R), ms_ps[:], work, OP.subtract)
                # rs = (var + eps)^-0.5 = exp(-0.5 * ln(var + eps))
                nc.scalar.activation(murs.bitcast(F32R), work, AF.Ln,
                                     bias=eps_sb[:])
                if masked:
                    nc.scalar.activation(rs.bitcast(F32R), murs, AF.Exp,
                                         scale=-0.5)
                    # fold attn scale + query mask into rs
                    nc.vector.tensor_tensor(
                        rs.bitcast(F32R), rs, msk_sb[:, cs], OP.mult)
                else:
                    nc.scalar.activation(rs.bitcast(F32R), murs, AF.Exp,
                                         scale=-0.5)
                # murs = -mu * rs
                nc.vector.scalar_tensor_tensor(
                    murs.bitcast(F32R), mu_sb, -1.0, rs, OP.mult, OP.mult)
                for t in range(NT):
                    rrep = ps_t.tile([P, CH], F32, name="pt", tag="pt")
                    _mm(nc, rrep[:], bcast_sb[:, t, :], rs.bitcast(F32R),
                        True, True)
                    mrep = ps_t.tile([P, CH], F32, name="pt", tag="pt")
                    _mm(nc, mrep[:], bcast_sb[:, t, :], murs.bitcast(F32R),
                        True, True)
                    nc.vector.tensor_tensor(
                        dst[t][:, cs].bitcast(F32R), dst[t][:, cs], rrep[:],
                        OP.mult)
                    nc.vector.tensor_tensor(
                        dst[t][:, cs].bitcast(F32R), dst[t][:, cs], mrep[:],
                        OP.add)
                if not masked:
                    # v projection reuses this chunk's kv x-tiles
                    for tl in range(CH // P):
                        tt = c * (CH // P) + tl
                        vp = ps_t.tile([P, CL], F32, name="pt", tag="pt")
                        for ct in range(CT):
                            _mm(nc, vp[:], xts[ct][:, tl * P:(tl + 1) * P],
                                wv_sb[:, ct, :], ct == 0, ct == CT - 1)
                        nc.vector.tensor_copy(
                            v_sb[:, tt, :, 0:HD].bitcast(F32R),
                            vp[:].rearrange("p (h d) -> p h d", h=HPG))

        for c in range(NCH):
            ln_chunk(qxT, wq_sb, q_sb, True, c)
            if c == 0:
                nc.sync.dma_start(
                    wk_sb[:], wk.rearrange("(ct p) m -> p ct m", p=P))
                nc.sync.dma_start(
                    wv_sb[:], wv.rearrange("(ct p) m -> p ct m", p=P))
                nc.sync.dma_start(v_sb[:, :, :, HD].bitcast(F32R), vones[:])
            ln_chunk(kvxT, wk_sb, k_sb, False, c)

    for pool in (st_pool, sq_pool, xkv_pool, xq_pool, w_pool, ps_t):
        pool.release()

    # ---------------- phase 2: attention + output projection ----------
    ps_s = tc.alloc_tile_pool(name="ps_s", bufs=2, space="PSUM")
    ps_o = tc.alloc_tile_pool(name="ps_o", bufs=2, space="PSUM")
    ps_t = tc.alloc_tile_pool(name="ps2", bufs=2, space="PSUM")
    wp_pool = tc.alloc_tile_pool(name="wp", bufs=1)
    e_pool = tc.alloc_tile_pool(name="e", bufs=4)
    o_pool = tc.alloc_tile_pool(name="o", bufs=2)
    rcp_pool = tc.alloc_tile_pool(name="rcp", bufs=2)
    out_pool = tc.alloc_tile_pool(name="ot", bufs=3)
    if True:
        wp_sb = wp_pool.tile([P, NT, C], F32R, name="wp", tag="wp")
        nc.sync.dma_start(wp_sb[:], wp.rearrange("(t p) m -> p t m", p=P))
        for qc in range(NCH):
            qs = slice(qc * CH, (qc + 1) * CH)
            o_t = [o_pool.tile([P, CH], F32, name=f"o{t}", tag=f"o{t}") for t in range(NT)]
            for h in range(HPG):
                t, off = h // 2, (h % 2) * HD
                po = ps_o.tile([HD + 1, CH], F32, name="po", tag="po")
                for kg in range(TT // KT_GRP):
                    sp = ps_s.tile([P, KT_GRP * CH], F32, name="sp", tag="sp")
                    for j in range(KT_GRP):
                        kt = kg * KT_GRP + j
                        _mm(nc, sp[:, j * CH:(j + 1) * CH],
                            k_sb[t][off:off + HD, kt * P:(kt + 1) * P].bitcast(F32R),
                            q_sb[t][off:off + HD, qs].bitcast(F32R),
                            True, True)
                    e = e_pool.tile([P, KT_GRP * CH], F32, name="e", tag="e")
                    nc.scalar.activation(e[:].bitcast(F32R), sp[:], AF.Exp)
                    for j in range(KT_GRP):
                        kt = kg * KT_GRP + j
                        _mm(nc, po[:], v_sb[:, kt, h, :].bitcast(F32R),
                            e[:, j * CH:(j + 1) * CH].bitcast(F32R),
                            kt == 0, kt == TT - 1)
                # stash denominator (po row 64) and raw O rows; the
                # normalize happens after the qc's batched reciprocal.
                db = 32 * (qc % 3)
                nc.vector.tensor_copy(
                    den_all[db:db + 1, h * CH:(h + 1) * CH].bitcast(F32R),
                    po[HD:HD + 1, :])
                nc.vector.tensor_copy(
                    o_t[t][off:off + HD, :].bitcast(F32R), po[0:HD, :])
            # batched reciprocal for all 6 heads of this qc: repack the
            # [1, 6*CH] denominator row into [32, 96] (DVE reciprocal cost
            # scales with free size only), invert, and scatter back.
            db = 32 * (qc % 3)
            dpk = rcp_pool.tile([32, HPG * CH // 32], F32, name="dpk", tag="dpk")
            nc.sync.dma_start(dpk[:], den_all[db:db + 1, :])
            rpk = rcp_pool.tile([32, HPG * CH // 32], F32, name="rpk", tag="rpk")
            nc.vector.reciprocal(rpk[:], dpk[:])
            nc.sync.dma_start(
                den_all[db:db + 1, :].bitcast(F32R), rpk[:].bitcast(F32R))
            for h in range(HPG):
                t, off = h // 2, (h % 2) * HD
                rrep = ps_t.tile([HD, CH], F32, name="pt", tag="pt")
                _mm(nc, rrep[:], ones4_sb[db:db + 1, :],
                    den_all[db:db + 1, h * CH:(h + 1) * CH].bitcast(F32R),
                    True, True)
                nc.vector.tensor_tensor(
                    o_t[t][off:off + HD, :].bitcast(F32R),
                    o_t[t][off:off + HD, :], rrep[:], OP.mult)
            for m in range(C // P):
                pp = ps_t.tile([P, CH], F32, name="pt", tag="pt")
                for t in range(NT):
                    _mm(nc, pp[:], wp_sb[:, t, m * P:(m + 1) * P],
                        o_t[t][:].bitcast(F32R), t == 0, t == NT - 1)
                ot = out_pool.tile([P, CH], F32, name="ot", tag="ot")
                nc.vector.tensor_copy(ot[:], pp[:])
                nc.sync.dma_start(outT[m * P:(m + 1) * P, qs], ot[:])

    for pool in (out_pool, rcp_pool, o_pool, e_pool, wp_pool,
                 ps_t, ps_o, ps_s, bpool, cpool):
        pool.release()


def build_bass():
    nc = bass.Bass(trn_type="TRN2", debug=False, num_devices=NCORES)
    qxT = nc.dram_tensor("qxT", [C, N], F32R, kind="ExternalInput").ap()
    kvxT = nc.dram_tensor("kvxT", [C, N], F32R, kind="ExternalInput").ap()
    wq = nc.dram_tensor("wq", [C, CL], F32R, kind="ExternalInput").ap()
    wk = nc.dram_tensor("wk", [C, CL], F32R, kind="ExternalInput").ap()
    wv = nc.dram_tensor("wv", [C, CL], F32R, kind="ExternalInput").ap()
    wp = nc.dram_tensor("wp", [CL, C], F32R, kind="ExternalInput").ap()
    msk = nc.dram_tensor("msk", [HPG, N], F32, kind="ExternalInput").ap()
    colsel = nc.dram_tensor("colsel", [P, NT, HPG], F32R,
                            kind="ExternalInput").ap()
    bcast = nc.dram_tensor("bcast", [HPG, NT, P], F32R,
                           kind="ExternalInput").ap()
    ones1 = nc.dram_tensor("ones1", [65, HD], F32R, kind="ExternalInput").ap()
    vones = nc.dram_tensor("vones", [P, TT, HPG], F32R,
                           kind="ExternalInput").ap()
    outT = nc.dram_tensor("outT", [C, N], F32, kind="ExternalOutput").ap()
    aps = (qxT, kvxT, wq, wk, wv, wp, msk, colsel, bcast, ones1, vones, outT)
    with _FixedTileContext(nc) as tc:
        _body(tc, aps)
    return nc


def make_in_maps(q_x, kv_x, attn_mask, Wq, Wkv, Wp):
    colsel = np.zeros((P, NT, HPG), np.float32)
    bcast = np.zeros((HPG, NT, P), np.float32)
    for t in range(NT):
        for pp in range(P):
            colsel[pp, t, 2 * t + pp // HD] = 1.0 / HD
            bcast[2 * t + pp // HD, t, pp] = 1.0
    ones1 = np.zeros((65, HD), np.float32)
    ones1[[0, 32, 64], :] = 1.0

    in_maps = []
    for core in range(NCORES):
        b, g = core // G, core % G
        sl = slice(g * CL, (g + 1) * CL)
        in_maps.append({
            "qxT": np.ascontiguousarray(q_x[b].T),
            "kvxT": np.ascontiguousarray(kv_x[b].T),
            "wq": np.ascontiguousarray(Wq[sl].T),
            "wk": np.ascontiguousarray(Wkv[sl].T),
            "wv": np.ascontiguousarray(Wkv[C + g * CL:C + (g + 1) * CL].T),
            "wp": np.ascontiguousarray(Wp[:, sl].T),
            "msk": np.broadcast_to(
                attn_mask[b].astype(np.float32) * SCALE, (HPG, N)).copy(),
            "colsel": colsel,
            "bcast": bcast,
            "ones1": ones1,
            "vones": np.ones((P, TT, HPG), np.float32),
        })
    return in_maps


_NC_CACHE = []


def get_nc():
    if not _NC_CACHE:
        _NC_CACHE.append(build_bass())
    return _NC_CACHE[0]


def kernel(q_x, kv_x, attn_mask, Wq, Wkv, qn_w, qn_b, kn_w, kn_b, Wp, bp,
           _profile=None):
    q_x = np.asarray(q_x, np.float32)
    kv_x = np.asarray(kv_x, np.float32)
    attn_mask = np.asarray(attn_mask)
    Wq = np.asarray(Wq, np.float32)
    Wkv = np.asarray(Wkv, np.float32)
    Wp = np.asarray(Wp, np.float32)
    bp = np.asarray(bp, np.float32)
    if not (np.all(np.asarray(qn_w) == 1) and np.all(np.asarray(qn_b) == 0)
            and np.all(np.asarray(kn_w) == 1) and np.all(np.asarray(kn_b) == 0)):
        raise NotImplementedError("kernel specialized to identity q/k norms")

    nc = get_nc()
    in_maps = make_in_maps(q_x, kv_x, attn_mask, Wq, Wkv, Wp)
    res = bass_utils.run_bass_kernel_spmd(
        nc, in_maps, core_ids=list(range(NCORES)))
    if _profile is not None:
        _profile.append(res)
    out = np.empty((B, N, C), np.float32)
    for b in range(B):
        acc = res.results[G * b]["outT"] + res.results[G * b + 1]["outT"]
        out[b] = acc.T + bp
    return out



# revision 3
# speedup vs baseline: 1.5323x; 1.5323x over previous
"""Cross-attention (B=4, N=2048, C=768, H=12, HD=64) on 8 TRN2 NeuronCores.

Sharding: core = (batch, head_group), 4 batches x 2 groups of 6 heads.
Each core computes its group's Q/K/V projections, per-head-dim LayerNorm,
attention, and a partial output projection; the host sums the two group
partials per batch and adds the bias.

Key optimizations over the f32r baseline:
 - The attn_mask masks whole QUERY rows; a masked query's output is exactly
   mean(v) @ Wp + bp, computed on the host in numpy.  The device only sees
   the gathered unmasked queries (NQ ~= 1060 of 2048), cutting S/PV/exp/
   q-proj/out-proj work by ~2x combined with the next point.
 - All matmuls run in fp16: 1 column/cycle streaming (fp32 runs HIGH/LOW
   two-pass) and FastWeightLoad hides LDWEIGHTS (disabled for fp32).
 - S = k~.T @ q~ contracts over HD=64 only, so head pairs are packed into
   the 128-row PE array with tile_position row tiling (auto-derived from
   base_partition 0/64) and run concurrently: S cost halves.
 - LN mean/meansq matmuls are column-tiled (out partitions 0-5 / 32-37 of
   one PSUM tile), the rs/murs broadcast matmuls are row-tiled (partitions
   0-5 / 32-37) - each pair runs concurrently.
 - Softmax needs no row max: LN bounds |S| <= 8, so exp(S) in [3e-4, 3e3],
   safely inside fp16/fp32 range.  The denominator rides along as a ones
   column appended to v (PV out row 64).  O is scaled by 2^-6 before the
   fp16 copy to dodge overflow; the reciprocal uses the same scaled den.
PSUM budget (8 banks): spA+spB (2+2) + poA+poB (1+1) + mix pp (2).
"""

import math

import numpy as np

import concourse.bass as bass
import concourse.mybir as mybir
from concourse import tile
from concourse import bass_utils
from concourse.tile_scheduler import N_PROCS
from concourse.vector_clock import ScopedClock, VectorClock

F32 = mybir.dt.float32
F16 = mybir.dt.float16
AF = mybir.ActivationFunctionType
OP = mybir.AluOpType

B, N, C, H, HD = 4, 2048, 768, 12, 64
G = 2                 # head groups (tensor parallel)
HPG = H // G          # 6 heads per group
CL = HPG * HD         # 384 local channels
P = 128
NT = CL // P          # 3 tiles of local channels
CT = C // P           # 6 contraction tiles
TT = N // P           # 16 key-token tiles
KCH = 512             # kv chunk size
EPS = 1e-5
SCALE = HD ** -0.5
LNB = -0.5 * math.log(HD)   # ln(SCALE): folded into the q-LN exp bias
OSHIFT = 2.0 ** -6          # pre-normalize scale to keep |o| in fp16 range
NCORES = 8

_nop_ctr = [0]


class _FixedTileContext(tile.TileContext):
    """Workaround for a walrus build that allows at most ONE sync-wait per
    instruction: split multi-wait instructions into single-wait NoOps on the
    same engine, and emit the kernel-tail drain's waits as a nop chain."""

    def _split_multiwait(self, insts):
        out = []
        for inst in insts:
            si = getattr(inst, "sync_info", None)
            waits = list(si.on_wait) if si is not None and si.on_wait else []
            if len(waits) > 1:
                eng = inst.engine
                for w in waits[:-1]:
                    _nop_ctr[0] += 1
                    nop = mybir.InstNoOp(
                        name=f"I-waitsplit-{_nop_ctr[0]}", ins=[], outs=[]
                    )
                    nop.engine = eng
                    nop.sync_info = mybir.SyncInfo(on_wait=[w], on_update=[])
                    self.nc.register_instruction(nop)
                    out.append(nop)
                inst.sync_info = mybir.SyncInfo(
                    on_wait=[waits[-1]], on_update=list(si.on_update)
                )
            out.append(inst)
        return out

    def _lower_ordered_insts(self, ordered):
        ordered = {bb: self._split_multiwait(ins) for bb, ins in ordered.items()}
        super()._lower_ordered_insts(ordered)

    def _drain_and_barrier(self, tick_clock, wait_clock):
        gc = tick_clock.global_clock
        vals = [gc[p] for p in range(N_PROCS)]
        for p in [q for q, v in enumerate(vals) if v > 0]:
            partial = VectorClock(
                [vals[q] if q == p else 0 for q in range(N_PROCS)]
            )
            nop = self.nc.sync.nop(nofuse=True, hint="tail_drain_wait")
            wait_clock.add_sem_waits(nop.ins, ScopedClock({None: partial}))
        self.nc.sync.drain()
        self.nc.all_engine_barrier()
        assert self.sems is not None
        popped = self.nc._tile_sem_poison_stack.pop()
        assert popped is self._sem_poison
        self.nc.clear_and_free_semaphores(list(self.sems.allocated().values()))
        self.nc.all_engine_barrier()


def _mm(nc, out, lhsT, rhs, start, stop):
    nc.tensor.matmul(
        out, lhsT, rhs, start=start, stop=stop, skip_group_check=True
    )


def _chunks(total, size=512):
    out, o = [], 0
    while o < total:
        s = min(size, total - o)
        out.append((o, s))
        o += s
    return out


def _body(tc, aps, NQ):
    nc = tc.nc
    qxT, kvxT, wq, wk, wv, wp, colsel, bcast, vones, ones16, outT = aps
    qch = _chunks(NQ)
    kch = _chunks(N)

    cpool = tc.alloc_tile_pool(name="consts", bufs=1)
    bpool = tc.alloc_tile_pool(name="big", bufs=1)

    colsel_sb = cpool.tile([P, NT, HPG], F16, name="colsel", tag="colsel")
    nc.sync.dma_start(colsel_sb[:], colsel[:])
    bcast_sb = cpool.tile([38, NT, P], F16, name="bcast", tag="bcast")
    nc.sync.dma_start(bcast_sb[:], bcast[:])
    ones16_sb = cpool.tile([1, HD], F16, name="ones16", tag="ones16")
    nc.sync.dma_start(ones16_sb[:], ones16[:])
    eps_sb = cpool.tile([HPG, 1], F32, name="eps", tag="eps")
    nc.vector.memset(eps_sb[:], EPS)
    lnb_sb = cpool.tile([HPG, 1], F32, name="lnb", tag="lnb")
    nc.vector.memset(lnb_sb[:], LNB)

    q_sb = [bpool.tile([P, NQ], F16, name=f"q{t}", tag=f"q{t}") for t in range(NT)]
    k_sb = [bpool.tile([P, N], F16, name=f"k{t}", tag=f"k{t}") for t in range(NT)]
    v_sb = bpool.tile([P, TT, HPG, HD + 1], F16, name="v", tag="v")

    wq_sb = bpool.tile([P, CT, CL], F16, name="wq", tag="wq")
    nc.sync.dma_start(wq_sb[:], wq.rearrange("(ct p) m -> p ct m", p=P))
    wk_sb = bpool.tile([P, CT, CL], F16, name="wk", tag="wk")
    nc.sync.dma_start(wk_sb[:], wk.rearrange("(ct p) m -> p ct m", p=P))
    wv_sb = bpool.tile([P, CT, CL], F16, name="wv", tag="wv")
    nc.sync.dma_start(wv_sb[:], wv.rearrange("(ct p) m -> p ct m", p=P))
    wp_sb = bpool.tile([P, NT, C], F16, name="wp", tag="wp")
    nc.sync.dma_start(wp_sb[:], wp.rearrange("(t p) m -> p t m", p=P))
    nc.sync.dma_start(v_sb[:, :, :, HD], vones[:])

    # PSUM pools: spA(2) + spB(2) + poA(1) + poB(1) + pp(2x1) = 8 banks
    ps_sp = tc.alloc_tile_pool(name="ps_sp", bufs=1, space="PSUM")
    ps_po = tc.alloc_tile_pool(name="ps_po", bufs=1, space="PSUM")
    ps_mix = tc.alloc_tile_pool(name="ps_mix", bufs=2, space="PSUM")

    x_pool = tc.alloc_tile_pool(name="x", bufs=7)
    sq_pool = tc.alloc_tile_pool(name="sq", bufs=2)
    st_pool = tc.alloc_tile_pool(name="st", bufs=2)
    e_pool = tc.alloc_tile_pool(name="e", bufs=2)
    o_pool = tc.alloc_tile_pool(name="o", bufs=2)
    den_pool = tc.alloc_tile_pool(name="den", bufs=2)
    out_pool = tc.alloc_tile_pool(name="ot", bufs=3)

    def proj(xT, w_sb, dst, cs, ch, xtag):
        xts = []
        for ct in range(CT):
            xt = x_pool.tile([P, ch], F16, name="xt", tag=xtag)
            nc.sync.dma_start(xt[:], xT[ct * P:(ct + 1) * P, cs])
            xts.append(xt)
        for t in range(NT):
            pp = ps_mix.tile([P, ch], F32, name="pp", tag="pp")
            for ct in range(CT):
                _mm(nc, pp[:], w_sb[:, ct, t * P:(t + 1) * P], xts[ct][:],
                    ct == 0, ct == CT - 1)
            nc.vector.tensor_copy(dst[t][:, cs], pp[:])
        return xts

    def ln_tail(dst, cs, ch, is_q):
        # mean / mean-square via column-tiled matmuls into one PSUM tile
        mums = ps_po.tile([38, ch], F32, name="mums", tag="poA")
        for t in range(NT):
            sq = sq_pool.tile([P, ch], F16, name="sq", tag="sq")
            nc.vector.tensor_tensor(sq[:], dst[t][:, cs], dst[t][:, cs],
                                    OP.mult)
            _mm(nc, mums[0:HPG, :], colsel_sb[:, t, :], dst[t][:, cs],
                t == 0, t == NT - 1)
            _mm(nc, mums[32:38, :], colsel_sb[:, t, :], sq[:],
                t == 0, t == NT - 1)
        stf = st_pool.tile([HPG, 3 * ch], F32, name="stf", tag="stf")
        sth = st_pool.tile([38, ch], F16, name="sth", tag="sth")
        mu = stf[:, 0:ch]
        work = stf[:, ch:2 * ch]
        lnv = stf[:, 2 * ch:3 * ch]
        nc.vector.tensor_copy(mu, mums[0:HPG, :])
        nc.vector.scalar_tensor_tensor(work, mu, 1.0, mu, OP.mult, OP.mult)
        nc.vector.tensor_tensor(work, mums[32:38, :], work, OP.subtract)
        nc.scalar.activation(lnv, work, AF.Ln, bias=eps_sb[:])
        # rs = exp(-0.5*ln(var+eps) [+ ln(scale) for q]) ; murs = -mu*rs
        nc.scalar.activation(sth[0:HPG, :], lnv, AF.Exp, scale=-0.5,
                             bias=(lnb_sb[:] if is_q else 0.0))
        nc.vector.scalar_tensor_tensor(sth[32:38, :], mu, -1.0,
                                       sth[0:HPG, :], OP.mult, OP.mult)
        for t in range(NT):
            rr = ps_mix.tile([P, ch], F32, name="rr", tag="pp")
            _mm(nc, rr[:], bcast_sb[0:HPG, t, :], sth[0:HPG, :], True, True)
            mr = ps_po.tile([P, ch], F32, name="mr", tag="poB")
            _mm(nc, mr[:], bcast_sb[32:38, t, :], sth[32:38, :], True, True)
            nc.vector.tensor_tensor(dst[t][:, cs], dst[t][:, cs], rr[:],
                                    OP.mult)
            nc.vector.tensor_tensor(dst[t][:, cs], dst[t][:, cs], mr[:],
                                    OP.add)

    def kv_chunk(c):
        co, ch = kch[c]
        cs = slice(co, co + ch)
        xts = proj(kvxT, wk_sb, k_sb, cs, ch, "xk")
        for tl in range(ch // P):
            ttk = co // P + tl
            vp = ps_mix.tile([P, CL], F32, name="vp", tag="pp")
            for ct in range(CT):
                _mm(nc, vp[:], xts[ct][:, tl * P:(tl + 1) * P],
                    wv_sb[:, ct, :], ct == 0, ct == CT - 1)
            nc.vector.tensor_copy(
                v_sb[:, ttk, :, 0:HD],
                vp[:].rearrange("p (h d) -> p h d", h=HPG))
        ln_tail(k_sb, cs, ch, False)

    def q_chunk(i):
        qo, ch = qch[i]
        cs = slice(qo, qo + ch)
        proj(qxT, wq_sb, q_sb, cs, ch, "xq")
        ln_tail(q_sb, cs, ch, True)

    def attn_chunk(i):
        qo, ch = qch[i]
        qs = slice(qo, qo + ch)
        den = den_pool.tile([1, HPG * ch], F32, name="den", tag="den")
        o_t = [o_pool.tile([P, ch], F16, name=f"o{t}", tag=f"o{t}")
               for t in range(NT)]
        for t in range(NT):
            hA, hB = 2 * t, 2 * t + 1
            poA = ps_po.tile([HD + 1, ch], F32, name="poA", tag="poA")
            poB = ps_po.tile([HD + 1, ch], F32, name="poB", tag="poB")
            for g in range(TT // 2):
                spA = ps_sp.tile([P, 2 * ch], F32, name="spA", tag="spA")
                spB = ps_sp.tile([P, 2 * ch], F32, name="spB", tag="spB")
                for j in range(2):
                    kt = 2 * g + j
                    # row-tiled pair: head A in PE rows 0-63, head B in 64-127
                    _mm(nc, spA[:, j * ch:(j + 1) * ch],
                        k_sb[t][0:HD, kt * P:(kt + 1) * P],
                        q_sb[t][0:HD, qs], True, True)
                    _mm(nc, spB[:, j * ch:(j + 1) * ch],
                        k_sb[t][HD:P, kt * P:(kt + 1) * P],
                        q_sb[t][HD:P, qs], True, True)
                eA = e_pool.tile([P, 2 * ch], F16, name="eA", tag="eA")
                eB = e_pool.tile([P, 2 * ch], F16, name="eB", tag="eB")
                nc.scalar.activation(eA[:], spA[:], AF.Exp)
                nc.scalar.activation(eB[:], spB[:], AF.Exp)
                for j in range(2):
                    kt = 2 * g + j
                    _mm(nc, poA[:], v_sb[:, kt, hA, :],
                        eA[:, j * ch:(j + 1) * ch], kt == 0, kt == TT - 1)
                    _mm(nc, poB[:], v_sb[:, kt, hB, :],
                        eB[:, j * ch:(j + 1) * ch], kt == 0, kt == TT - 1)
            # stash scaled denominators and scaled raw O rows (fp16-safe)
            nc.vector.tensor_scalar_mul(
                den[0:1, hA * ch:(hA + 1) * ch], poA[HD:HD + 1, :], OSHIFT)
            nc.vector.tensor_scalar_mul(
                den[0:1, hB * ch:(hB + 1) * ch], poB[HD:HD + 1, :], OSHIFT)
            nc.vector.tensor_scalar_mul(o_t[t][0:HD, :], poA[0:HD, :], OSHIFT)
            nc.vector.tensor_scalar_mul(o_t[t][HD:P, :], poB[0:HD, :], OSHIFT)
        # batched reciprocal: repack [1, 6*ch] -> [32, 6*ch/32] (DVE
        # reciprocal cost scales with free size only), invert, unpack fp16
        w32 = HPG * ch // 32
        dpk = den_pool.tile([32, w32], F32, name="dpk", tag="dpk")
        nc.sync.dma_start(dpk[:], den[0:1, :])
        rpk = den_pool.tile([32, w32], F32, name="rpk", tag="rpk")
        nc.vector.reciprocal(rpk[:], dpk[:])
        rpk16 = den_pool.tile([32, w32], F16, name="rpk16", tag="rpk16")
        nc.vector.tensor_copy(rpk16[:], rpk[:])
        denr = den_pool.tile([1, HPG * ch], F16, name="denr", tag="denr")
        nc.sync.dma_start(denr[0:1, :], rpk16[:])
        for t in range(NT):
            for hh in range(2):
                h = 2 * t + hh
                rb = ps_mix.tile([HD, ch], F32, name="rb", tag="pp")
                _mm(nc, rb[:], ones16_sb[:],
                    denr[0:1, h * ch:(h + 1) * ch], True, True)
                nc.vector.tensor_tensor(
                    o_t[t][hh * HD:(hh + 1) * HD, :],
                    o_t[t][hh * HD:(hh + 1) * HD, :], rb[:], OP.mult)
        for m in range(CT):
            pp = ps_mix.tile([P, ch], F32, name="op", tag="pp")
            for t in range(NT):
                _mm(nc, pp[:], wp_sb[:, t, m * P:(m + 1) * P], o_t[t][:],
                    t == 0, t == NT - 1)
            ot = out_pool.tile([P, ch], F16, name="ot", tag="ot")
            nc.vector.tensor_copy(ot[:], pp[:])
            nc.sync.dma_start(outT[m * P:(m + 1) * P, qs], ot[:])

    for c in range(len(kch)):
        kv_chunk(c)
    q_chunk(0)
    for i in range(len(qch)):
        attn_chunk(i)
        if i + 1 < len(qch):
            q_chunk(i + 1)

    for pool in (out_pool, den_pool, o_pool, e_pool, st_pool, sq_pool,
                 x_pool, ps_mix, ps_po, ps_sp, bpool, cpool):
        pool.release()


def build_bass(NQ):
    nc = bass.Bass(trn_type="TRN2", debug=False, num_devices=NCORES)
    qxT = nc.dram_tensor("qxT", [C, NQ], F16, kind="ExternalInput").ap()
    kvxT = nc.dram_tensor("kvxT", [C, N], F16, kind="ExternalInput").ap()
    wq = nc.dram_tensor("wq", [C, CL], F16, kind="ExternalInput").ap()
    wk = nc.dram_tensor("wk", [C, CL], F16, kind="ExternalInput").ap()
    wv = nc.dram_tensor("wv", [C, CL], F16, kind="ExternalInput").ap()
    wp = nc.dram_tensor("wp", [CL, C], F16, kind="ExternalInput").ap()
    colsel = nc.dram_tensor("colsel", [P, NT, HPG], F16,
                            kind="ExternalInput").ap()
    bcast = nc.dram_tensor("bcast", [38, NT, P], F16,
                           kind="ExternalInput").ap()
    vones = nc.dram_tensor("vones", [P, TT, HPG], F16,
                           kind="ExternalInput").ap()
    ones16 = nc.dram_tensor("ones16", [1, HD], F16,
                            kind="ExternalInput").ap()
    outT = nc.dram_tensor("outT", [C, NQ], F16, kind="ExternalOutput").ap()
    aps = (qxT, kvxT, wq, wk, wv, wp, colsel, bcast, vones, ones16, outT)
    with _FixedTileContext(nc) as tc:
        _body(tc, aps, NQ)
    return nc


def make_in_maps(q_x, kv_x, attn_mask, Wq, Wkv, Wp, NQ, idxs):
    colsel = np.zeros((P, NT, HPG), np.float16)
    bcast = np.zeros((38, NT, P), np.float16)
    for t in range(NT):
        for pp in range(P):
            h = 2 * t + pp // HD
            colsel[pp, t, h] = 1.0 / HD
            bcast[h, t, pp] = 1.0
    bcast[32:38] = bcast[0:HPG]  # mirror for the row-tiled murs broadcast
    ones16 = np.ones((1, HD), np.float16)
    vones = np.ones((P, TT, HPG), np.float16)

    in_maps = []
    for core in range(NCORES):
        b, g = core // G, core % G
        sl = slice(g * CL, (g + 1) * CL)
        idx = idxs[b]
        pad = np.zeros(NQ, np.int64)
        pad[:len(idx)] = idx
        if len(idx) < NQ:
            pad[len(idx):] = idx[0] if len(idx) else 0
        in_maps.append({
            "qxT": np.ascontiguousarray(q_x[b][pad].T.astype(np.float16)),
            "kvxT": np.ascontiguousarray(kv_x[b].T.astype(np.float16)),
            "wq": np.ascontiguousarray(Wq[sl].T.astype(np.float16)),
            "wk": np.ascontiguousarray(Wkv[sl].T.astype(np.float16)),
            "wv": np.ascontiguousarray(
                Wkv[C + g * CL:C + (g + 1) * CL].T.astype(np.float16)),
            "wp": np.ascontiguousarray(Wp[:, sl].T.astype(np.float16)),
            "colsel": colsel,
            "bcast": bcast,
            "vones": vones,
            "ones16": ones16,
        })
    return in_maps


_NC_CACHE = {}


def get_nc(NQ):
    if NQ not in _NC_CACHE:
        _NC_CACHE[NQ] = build_bass(NQ)
    return _NC_CACHE[NQ]


def prepare(q_x, kv_x, attn_mask, Wq, Wkv, Wp):
    mask = np.asarray(attn_mask).astype(bool)
    idxs = [np.flatnonzero(mask[b]) for b in range(B)]
    numax = max(1, max(len(i) for i in idxs))
    NQ = ((numax + 31) // 32) * 32
    nc = get_nc(NQ)
    in_maps = make_in_maps(q_x, kv_x, mask, Wq, Wkv, Wp, NQ, idxs)
    return nc, in_maps, idxs


def kernel(q_x, kv_x, attn_mask, Wq, Wkv, qn_w, qn_b, kn_w, kn_b, Wp, bp,
           _profile=None):
    q_x = np.asarray(q_x, np.float32)
    kv_x = np.asarray(kv_x, np.float32)
    Wq = np.asarray(Wq, np.float32)
    Wkv = np.asarray(Wkv, np.float32)
    Wp = np.asarray(Wp, np.float32)
    bp = np.asarray(bp, np.float32)
    if not (np.all(np.asarray(qn_w) == 1) and np.all(np.asarray(qn_b) == 0)
            and np.all(np.asarray(kn_w) == 1) and np.all(np.asarray(kn_b) == 0)):
        raise NotImplementedError("kernel specialized to identity q/k norms")

    nc, in_maps, idxs = prepare(q_x, kv_x, attn_mask, Wq, Wkv, Wp)
    res = bass_utils.run_bass_kernel_spmd(
        nc, in_maps, core_ids=list(range(NCORES)))
    if _profile is not None:
        _profile.append(res)

    # masked-query rows: softmax over an all -1e9 row is uniform, so the
    # output is exactly mean_k(v) @ Wp.T + bp -- pure host math.
    vmean = kv_x.mean(axis=1) @ Wkv[C:].T          # [B, C]
    ymask = vmean @ Wp.T + bp                      # [B, C]
    out = np.empty((B, N, C), np.float32)
    for b in range(B):
        acc = (res.results[G * b]["outT"].astype(np.float32)
               + res.results[G * b + 1]["outT"].astype(np.float32))
        out[b] = ymask[b]
        nb = len(idxs[b])
        out[b, idxs[b]] = acc.T[:nb] + bp
    return out


# revision 7
# speedup vs baseline: 1.7016x; 1.1105x over previous
"""Cross-attention (B=4, N=2048, C=768, H=12, HD=64) on 8 TRN2 NeuronCores.

Sharding: core = (batch, head_group), 4 batches x 2 groups of 6 heads.
Each core computes its group's Q/K/V projections, per-head-dim LayerNorm,
attention, and a partial output projection; the host sums the two group
partials per batch and adds the bias.

Key optimizations over the f32r baseline:
 - The attn_mask masks whole QUERY rows; a masked query's output is exactly
   mean(v) @ Wp + bp, computed on the host in numpy.  The device only sees
   the gathered unmasked queries (NQ ~= 1060 of 2048), cutting S/PV/exp/
   q-proj/out-proj work by ~2x combined with the next point.
 - All matmuls run in fp16: 1 column/cycle streaming (fp32 runs HIGH/LOW
   two-pass) and FastWeightLoad hides LDWEIGHTS (disabled for fp32).
 - S = k~.T @ q~ contracts over HD=64 only, so head pairs are packed into
   the 128-row PE array with tile_position row tiling (auto-derived from
   base_partition 0/64) and run concurrently: S cost halves.
 - LN mean/meansq matmuls are column-tiled (out partitions 0-5 / 32-37 of
   one PSUM tile), the rs/murs broadcast matmuls are row-tiled (partitions
   0-5 / 32-37) - each pair runs concurrently.
 - Softmax needs no row max: LN bounds |S| <= 8, so exp(S) in [3e-4, 3e3],
   safely inside fp16/fp32 range.  The denominator rides along as a ones
   column appended to v (PV out row 64).  O is scaled by 2^-6 before the
   fp16 copy to dodge overflow; the reciprocal uses the same scaled den.
PSUM budget (8 banks): spA+spB (2+2) + poA+poB (1+1) + mix pp (2).
"""

import math

import numpy as np

import concourse.bass as bass
import concourse.mybir as mybir
from concourse import tile
from concourse import bass_utils
from concourse.tile_scheduler import N_PROCS
from concourse.vector_clock import ScopedClock, VectorClock

F32 = mybir.dt.float32
F16 = mybir.dt.float16
AF = mybir.ActivationFunctionType
OP = mybir.AluOpType

B, N, C, H, HD = 4, 2048, 768, 12, 64
G = 2                 # head groups (tensor parallel)
HPG = H // G          # 6 heads per group
CL = HPG * HD         # 384 local channels
P = 128
NT = CL // P          # 3 tiles of local channels
CT = C // P           # 6 contraction tiles
TT = N // P           # 16 key-token tiles
KCH = 512             # kv chunk size
EPS = 1e-5
SCALE = HD ** -0.5
LNB = -0.5 * math.log(HD)   # ln(SCALE): folded into the q-LN exp bias
OSHIFT = 2.0 ** -6          # pre-normalize scale to keep |o| in fp16 range
NCORES = 8

_nop_ctr = [0]


class _FixedTileContext(tile.TileContext):
    """Workaround for a walrus build that allows at most ONE sync-wait per
    instruction: split multi-wait instructions into single-wait NoOps on the
    same engine, and emit the kernel-tail drain's waits as a nop chain."""

    def _split_multiwait(self, insts):
        out = []
        for inst in insts:
            si = getattr(inst, "sync_info", None)
            waits = list(si.on_wait) if si is not None and si.on_wait else []
            if len(waits) > 1:
                eng = inst.engine
                for w in waits[:-1]:
                    _nop_ctr[0] += 1
                    nop = mybir.InstNoOp(
                        name=f"I-waitsplit-{_nop_ctr[0]}", ins=[], outs=[]
                    )
                    nop.engine = eng
                    nop.sync_info = mybir.SyncInfo(on_wait=[w], on_update=[])
                    self.nc.register_instruction(nop)
                    out.append(nop)
                inst.sync_info = mybir.SyncInfo(
                    on_wait=[waits[-1]], on_update=list(si.on_update)
                )
            out.append(inst)
        return out

    def _lower_ordered_insts(self, ordered):
        ordered = {bb: self._split_multiwait(ins) for bb, ins in ordered.items()}
        super()._lower_ordered_insts(ordered)

    def _drain_and_barrier(self, tick_clock, wait_clock):
        gc = tick_clock.global_clock
        vals = [gc[p] for p in range(N_PROCS)]
        for p in [q for q, v in enumerate(vals) if v > 0]:
            partial = VectorClock(
                [vals[q] if q == p else 0 for q in range(N_PROCS)]
            )
            nop = self.nc.sync.nop(nofuse=True, hint="tail_drain_wait")
            wait_clock.add_sem_waits(nop.ins, ScopedClock({None: partial}))
        self.nc.sync.drain()
        self.nc.all_engine_barrier()
        assert self.sems is not None
        popped = self.nc._tile_sem_poison_stack.pop()
        assert popped is self._sem_poison
        self.nc.clear_and_free_semaphores(list(self.sems.allocated().values()))
        self.nc.all_engine_barrier()


def _mm(nc, out, lhsT, rhs, start, stop):
    nc.tensor.matmul(
        out, lhsT, rhs, start=start, stop=stop, skip_group_check=True
    )


def _chunks(total, size=512):
    out, o = [], 0
    while o < total:
        s = min(size, total - o)
        out.append((o, s))
        o += s
    return out


def _step(fill):
    """Advance the round-robin fill queue by one piece (if any)."""
    while fill:
        gen = fill[0]
        try:
            next(gen)
            fill.rotate(-1)
            return
        except StopIteration:
            fill.popleft()


def _drain(gen):
    for _ in gen:
        pass


def _body(tc, aps, NQ):
    import collections
    nc = tc.nc
    qxT, kvxT, wq, wk, wv, wp, colsel, bcast, vones, ones16, outT = aps
    qch = _chunks(NQ)
    kch = _chunks(N)

    cpool = tc.alloc_tile_pool(name="consts", bufs=1)
    bpool = tc.alloc_tile_pool(name="big", bufs=1)

    colsel_sb = cpool.tile([P, NT, HPG], F16, name="colsel", tag="colsel")
    nc.sync.dma_start(colsel_sb[:], colsel[:])
    bcast_sb = cpool.tile([38, NT, P], F16, name="bcast", tag="bcast")
    nc.sync.dma_start(bcast_sb[:], bcast[:])
    ones16_sb = cpool.tile([1, HD], F16, name="ones16", tag="ones16")
    nc.sync.dma_start(ones16_sb[:], ones16[:])
    eps_sb = cpool.tile([HPG, 1], F32, name="eps", tag="eps")
    nc.vector.memset(eps_sb[:], EPS)
    lnb_sb = cpool.tile([HPG, 1], F32, name="lnb", tag="lnb")
    nc.vector.memset(lnb_sb[:], LNB)

    q_sb = [bpool.tile([P, NQ], F16, name=f"q{t}", tag=f"q{t}") for t in range(NT)]
    k_sb = [bpool.tile([P, N], F16, name=f"k{t}", tag=f"k{t}") for t in range(NT)]
    v_sb = bpool.tile([P, TT, HPG, HD + 1], F16, name="v", tag="v")

    wq_sb = bpool.tile([P, CT, CL], F16, name="wq", tag="wq")
    nc.sync.dma_start(wq_sb[:], wq.rearrange("(ct p) m -> p ct m", p=P))
    wk_sb = bpool.tile([P, CT, CL], F16, name="wk", tag="wk")
    nc.sync.dma_start(wk_sb[:], wk.rearrange("(ct p) m -> p ct m", p=P))
    wv_sb = bpool.tile([P, CT, CL], F16, name="wv", tag="wv")
    nc.sync.dma_start(wv_sb[:], wv.rearrange("(ct p) m -> p ct m", p=P))
    wp_sb = bpool.tile([P, NT, C], F16, name="wp", tag="wp")
    nc.sync.dma_start(wp_sb[:], wp.rearrange("(t p) m -> p t m", p=P))
    nc.sync.dma_start(v_sb[:, :, :, HD], vones[:])

    # PSUM pools: spA(2) + spB(2) + poA(1) + poB(1) + pp(2x1) = 8 banks
    ps_sp = tc.alloc_tile_pool(name="ps_sp", bufs=1, space="PSUM")
    ps_po = tc.alloc_tile_pool(name="ps_po", bufs=1, space="PSUM")
    ps_mix = tc.alloc_tile_pool(name="ps_mix", bufs=2, space="PSUM")

    x_pool = tc.alloc_tile_pool(name="x", bufs=6)
    sq_pool = tc.alloc_tile_pool(name="sq", bufs=2)
    st_pool = tc.alloc_tile_pool(name="st", bufs=2)
    e_pool = tc.alloc_tile_pool(name="e", bufs=2)
    o_pool = tc.alloc_tile_pool(name="o", bufs=2)
    den_pool = tc.alloc_tile_pool(name="den", bufs=2)
    out_pool = tc.alloc_tile_pool(name="ot", bufs=3)

    nq = len(qch)
    NKB = len(kch)                       # 4 k-blocks of 4 k-tiles each
    KPB = KCH // P                       # 4 k-tiles per block
    # flash accumulators: [65, ch] fp32 per (q-chunk, tile, head)
    o_acc = {}
    for qi, (qo, ch) in enumerate(qch):
        for t in range(NT):
            for hh in range(2):
                o_acc[(qi, t, hh)] = bpool.tile(
                    [HD + 1, ch], F32, name=f"oa{qi}_{t}_{hh}",
                    tag=f"oa{qi}_{t}_{hh}")

    def ln_gen(dst, cs, ch, is_q):
        """Projection LN tail: mean/meansq (column-tiled), rs/murs, apply
        (row-tiled broadcast matmuls).  All PSUM via the 'pp' tag."""
        mums = ps_mix.tile([38, ch], F32, name="mums", tag="pp")
        for t in range(NT):
            sq = sq_pool.tile([P, ch], F16, name="sq", tag="sq")
            nc.vector.tensor_tensor(sq[:], dst[t][:, cs], dst[t][:, cs],
                                    OP.mult)
            _mm(nc, mums[0:HPG, :], colsel_sb[:, t, :], dst[t][:, cs],
                t == 0, t == NT - 1)
            _mm(nc, mums[32:38, :], colsel_sb[:, t, :], sq[:],
                t == 0, t == NT - 1)
        yield
        stf = st_pool.tile([HPG, 3 * ch], F32, name="stf", tag="stf")
        sth = st_pool.tile([38, ch], F16, name="sth", tag="sth")
        mu = stf[:, 0:ch]
        work = stf[:, ch:2 * ch]
        lnv = stf[:, 2 * ch:3 * ch]
        nc.vector.tensor_copy(mu, mums[0:HPG, :])
        nc.vector.scalar_tensor_tensor(work, mu, 1.0, mu, OP.mult, OP.mult)
        nc.vector.tensor_tensor(work, mums[32:38, :], work, OP.subtract)
        nc.scalar.activation(lnv, work, AF.Ln, bias=eps_sb[:])
        # rs = exp(-0.5*ln(var+eps) [+ ln(scale) for q]) ; murs = -mu*rs
        nc.scalar.activation(sth[0:HPG, :], lnv, AF.Exp, scale=-0.5,
                             bias=(lnb_sb[:] if is_q else 0.0))
        nc.vector.scalar_tensor_tensor(sth[32:38, :], mu, -1.0,
                                       sth[0:HPG, :], OP.mult, OP.mult)
        yield
        for t in range(NT):
            rr = ps_mix.tile([P, ch], F32, name="rr", tag="pp")
            _mm(nc, rr[:], bcast_sb[0:HPG, t, :], sth[0:HPG, :], True, True)
            mr = ps_mix.tile([P, ch], F32, name="mr", tag="pp")
            _mm(nc, mr[:], bcast_sb[32:38, t, :], sth[32:38, :], True, True)
            nc.vector.tensor_tensor(dst[t][:, cs], dst[t][:, cs], rr[:],
                                    OP.mult)
            nc.vector.tensor_tensor(dst[t][:, cs], dst[t][:, cs], mr[:],
                                    OP.add)
            yield

    def kv_gen(c):
        co, ch = kch[c]
        cs = slice(co, co + ch)
        xts = []
        for ct in range(CT):
            xt = x_pool.tile([P, ch], F16, name="xt", tag=f"xk{c}")
            nc.sync.dma_start(xt[:], kvxT[ct * P:(ct + 1) * P, cs])
            xts.append(xt)
        yield
        for t in range(NT):
            pp = ps_mix.tile([P, ch], F32, name="pp", tag="pp")
            for ct in range(CT):
                _mm(nc, pp[:], wk_sb[:, ct, t * P:(t + 1) * P], xts[ct][:],
                    ct == 0, ct == CT - 1)
            nc.vector.tensor_copy(k_sb[t][:, cs], pp[:])
            yield
        for tl in range(ch // P):
            ttk = co // P + tl
            vp = ps_mix.tile([P, CL], F32, name="vp", tag="pp")
            for ct in range(CT):
                _mm(nc, vp[:], xts[ct][:, tl * P:(tl + 1) * P],
                    wv_sb[:, ct, :], ct == 0, ct == CT - 1)
            nc.vector.tensor_copy(
                v_sb[:, ttk, :, 0:HD],
                vp[:].rearrange("p (h d) -> p h d", h=HPG))
            yield
        yield from ln_gen(k_sb, cs, ch, False)

    def q_gen(i):
        qo, ch = qch[i]
        cs = slice(qo, qo + ch)
        xts = []
        for ct in range(CT):
            xt = x_pool.tile([P, ch], F16, name="xt", tag=f"xq{i}")
            nc.sync.dma_start(xt[:], qxT[ct * P:(ct + 1) * P, cs])
            xts.append(xt)
        yield
        for t in range(NT):
            pp = ps_mix.tile([P, ch], F32, name="pp", tag="pp")
            for ct in range(CT):
                _mm(nc, pp[:], wq_sb[:, ct, t * P:(t + 1) * P], xts[ct][:],
                    ct == 0, ct == CT - 1)
            nc.vector.tensor_copy(q_sb[t][:, cs], pp[:])
            yield
        yield from ln_gen(q_sb, cs, ch, True)

    def attn_unit(qi, t, blk, fill):
        """S + exp + PV for one (q-chunk, head-pair) over one k-block
        (4 k-tiles), accumulating into o_acc via DVE."""
        qo, ch = qch[qi]
        qs = slice(qo, qo + ch)
        hA, hB = 2 * t, 2 * t + 1
        poA = ps_po.tile([HD + 1, ch], F32, name="poA", tag="poA")
        poB = ps_po.tile([HD + 1, ch], F32, name="poB", tag="poB")
        for g in range(KPB // 2):
            spA = ps_sp.tile([P, 2 * ch], F32, name="spA", tag="spA")
            spB = ps_sp.tile([P, 2 * ch], F32, name="spB", tag="spB")
            for j in range(2):
                kt = blk * KPB + 2 * g + j
                # row-tiled pair: head A in PE rows 0-63, head B in 64-127
                _mm(nc, spA[:, j * ch:(j + 1) * ch],
                    k_sb[t][0:HD, kt * P:(kt + 1) * P],
                    q_sb[t][0:HD, qs], True, True)
                _mm(nc, spB[:, j * ch:(j + 1) * ch],
                    k_sb[t][HD:P, kt * P:(kt + 1) * P],
                    q_sb[t][HD:P, qs], True, True)
            eA = e_pool.tile([P, 2 * ch], F16, name="eA", tag="eA")
            eB = e_pool.tile([P, 2 * ch], F16, name="eB", tag="eB")
            nc.scalar.activation(eA[:], spA[:], AF.Exp)
            nc.scalar.activation(eB[:], spB[:], AF.Exp)
            for j in range(2):
                lk = 2 * g + j
                kt = blk * KPB + lk
                _mm(nc, poA[:], v_sb[:, kt, hA, :],
                    eA[:, j * ch:(j + 1) * ch], lk == 0, lk == KPB - 1)
                _mm(nc, poB[:], v_sb[:, kt, hB, :],
                    eB[:, j * ch:(j + 1) * ch], lk == 0, lk == KPB - 1)
            _step(fill)
        accA, accB = o_acc[(qi, t, 0)], o_acc[(qi, t, 1)]
        if blk == 0:
            nc.vector.tensor_copy(accA[:], poA[:])
            nc.vector.tensor_copy(accB[:], poB[:])
        else:
            nc.vector.tensor_tensor(accA[:], accA[:], poA[:], OP.add)
            nc.vector.tensor_tensor(accB[:], accB[:], poB[:], OP.add)

    def tail_gen(qi):
        """Normalize by the softmax denominator and project out."""
        qo, ch = qch[qi]
        qs = slice(qo, qo + ch)
        den = den_pool.tile([1, HPG * ch], F32, name="den", tag="den")
        for t in range(NT):
            for hh in range(2):
                h = 2 * t + hh
                nc.vector.tensor_scalar_mul(
                    den[0:1, h * ch:(h + 1) * ch],
                    o_acc[(qi, t, hh)][HD:HD + 1, :], OSHIFT)
        # batched reciprocal: repack [1, 6*ch] -> [32, 6*ch/32] (DVE
        # reciprocal cost scales with free size only), invert, unpack fp16
        w32 = HPG * ch // 32
        dpk = den_pool.tile([32, w32], F32, name="dpk", tag="dpk")
        nc.sync.dma_start(dpk[:], den[0:1, :])
        rpk = den_pool.tile([32, w32], F32, name="rpk", tag="rpk")
        nc.vector.reciprocal(rpk[:], dpk[:])
        rpk16 = den_pool.tile([32, w32], F16, name="rpk16", tag="rpk16")
        nc.vector.tensor_copy(rpk16[:], rpk[:])
        denr = den_pool.tile([1, HPG * ch], F16, name="denr", tag="denr")
        nc.sync.dma_start(denr[0:1, :], rpk16[:])
        yield
        o_t = [o_pool.tile([P, ch], F16, name=f"o{t}", tag=f"o{t}")
               for t in range(NT)]
        for t in range(NT):
            for hh in range(2):
                h = 2 * t + hh
                rb = ps_mix.tile([HD, ch], F32, name="rb", tag="pp")
                _mm(nc, rb[:], ones16_sb[:],
                    denr[0:1, h * ch:(h + 1) * ch], True, True)
                # o = (acc * 2^-6) * (1 / (den * 2^-6)) -- fp16-safe
                nc.vector.scalar_tensor_tensor(
                    o_t[t][hh * HD:(hh + 1) * HD, :],
                    o_acc[(qi, t, hh)][0:HD, :], OSHIFT, rb[:],
                    OP.mult, OP.mult)
            yield
        for m in range(CT):
            pp = ps_mix.tile([P, ch], F32, name="op", tag="pp")
            for t in range(NT):
                _mm(nc, pp[:], wp_sb[:, t, m * P:(m + 1) * P], o_t[t][:],
                    t == 0, t == NT - 1)
            ot = out_pool.tile([P, ch], F16, name="ot", tag="ot")
            nc.vector.tensor_copy(ot[:], pp[:])
            nc.sync.dma_start(outT[m * P:(m + 1) * P, qs], ot[:])
            yield

    # ---- schedule ----------------------------------------------------
    # Head: kv blocks 0-1 and q chunk 0 run dense (PE-bound, ACT idle).
    # Attention (exp/ACT-bound) then runs block-major with the remaining
    # projection work fed into the PE's idle slots so the HAM clock gate
    # never sees an idle PE window: block0 <- q chunks 1+, block1 <- kv2,
    # block2 <- kv3, block3 <- per-chunk output tails.
    _drain(kv_gen(0))
    _drain(kv_gen(1))
    _drain(q_gen(0))
    kvg = {2: kv_gen(2), 3: kv_gen(3)}
    qgens = {i: q_gen(i) for i in range(1, nq)}
    fill = collections.deque(qgens.values())
    for blk in range(NKB):
        # producers must be fully issued before their consumers (the tile
        # framework orders by issue): force-drain whatever the fill queue
        # hasn't finished by the time it's needed.
        if blk == 1 and 2 in kvg:
            fill.append(kvg[2])
        elif blk == 2:
            _drain(kvg[2])
            fill.append(kvg[3])
        elif blk == 3:
            _drain(kvg[3])
        for qi in range(nq):
            if blk == 0 and qi in qgens:
                _drain(qgens[qi])
            for t in range(NT):
                attn_unit(qi, t, blk, fill)
            if blk == NKB - 1:
                fill.append(tail_gen(qi))
    while fill:
        _step(fill)

    for pool in (out_pool, den_pool, o_pool, e_pool, st_pool, sq_pool,
                 x_pool, ps_mix, ps_po, ps_sp, bpool, cpool):
        pool.release()


def build_bass(NQ):
    nc = bass.Bass(trn_type="TRN2", debug=False, num_devices=NCORES)
    qxT = nc.dram_tensor("qxT", [C, NQ], F16, kind="ExternalInput").ap()
    kvxT = nc.dram_tensor("kvxT", [C, N], F16, kind="ExternalInput").ap()
    wq = nc.dram_tensor("wq", [C, CL], F16, kind="ExternalInput").ap()
    wk = nc.dram_tensor("wk", [C, CL], F16, kind="ExternalInput").ap()
    wv = nc.dram_tensor("wv", [C, CL], F16, kind="ExternalInput").ap()
    wp = nc.dram_tensor("wp", [CL, C], F16, kind="ExternalInput").ap()
    colsel = nc.dram_tensor("colsel", [P, NT, HPG], F16,
                            kind="ExternalInput").ap()
    bcast = nc.dram_tensor("bcast", [38, NT, P], F16,
                           kind="ExternalInput").ap()
    vones = nc.dram_tensor("vones", [P, TT, HPG], F16,
                           kind="ExternalInput").ap()
    ones16 = nc.dram_tensor("ones16", [1, HD], F16,
                            kind="ExternalInput").ap()
    outT = nc.dram_tensor("outT", [C, NQ], F16, kind="ExternalOutput").ap()
    aps = (qxT, kvxT, wq, wk, wv, wp, colsel, bcast, vones, ones16, outT)
    with _FixedTileContext(nc) as tc:
        _body(tc, aps, NQ)
    return nc


def make_in_maps(q_x, kv_x, attn_mask, Wq, Wkv, Wp, NQ, idxs):
    colsel = np.zeros((P, NT, HPG), np.float16)
    bcast = np.zeros((38, NT, P), np.float16)
    for t in range(NT):
        for pp in range(P):
            h = 2 * t + pp // HD
            colsel[pp, t, h] = 1.0 / HD
            bcast[h, t, pp] = 1.0
    bcast[32:38] = bcast[0:HPG]  # mirror for the row-tiled murs broadcast
    ones16 = np.ones((1, HD), np.float16)
    vones = np.ones((P, TT, HPG), np.float16)

    in_maps = []
    for core in range(NCORES):
        b, g = core // G, core % G
        sl = slice(g * CL, (g + 1) * CL)
        idx = idxs[b]
        pad = np.zeros(NQ, np.int64)
        pad[:len(idx)] = idx
        if len(idx) < NQ:
            pad[len(idx):] = idx[0] if len(idx) else 0
        in_maps.append({
            "qxT": np.ascontiguousarray(q_x[b][pad].T.astype(np.float16)),
            "kvxT": np.ascontiguousarray(kv_x[b].T.astype(np.float16)),
            "wq": np.ascontiguousarray(Wq[sl].T.astype(np.float16)),
            "wk": np.ascontiguousarray(Wkv[sl].T.astype(np.float16)),
            "wv": np.ascontiguousarray(
                Wkv[C + g * CL:C + (g + 1) * CL].T.astype(np.float16)),
            "wp": np.ascontiguousarray(Wp[:, sl].T.astype(np.float16)),
            "colsel": colsel,
            "bcast": bcast,
            "vones": vones,
            "ones16": ones16,
        })
    return in_maps


_NC_CACHE = {}


def get_nc(NQ):
    if NQ not in _NC_CACHE:
        _NC_CACHE[NQ] = build_bass(NQ)
    return _NC_CACHE[NQ]


def prepare(q_x, kv_x, attn_mask, Wq, Wkv, Wp):
    mask = np.asarray(attn_mask).astype(bool)
    idxs = [np.flatnonzero(mask[b]) for b in range(B)]
    numax = max(1, max(len(i) for i in idxs))
    NQ = ((numax + 31) // 32) * 32
    nc = get_nc(NQ)
    in_maps = make_in_maps(q_x, kv_x, mask, Wq, Wkv, Wp, NQ, idxs)
    return nc, in_maps, idxs


def kernel(q_x, kv_x, attn_mask, Wq, Wkv, qn_w, qn_b, kn_w, kn_b, Wp, bp,
           _profile=None):
    q_x = np.asarray(q_x, np.float32)
    kv_x = np.asarray(kv_x, np.float32)
    Wq = np.asarray(Wq, np.float32)
    Wkv = np.asarray(Wkv, np.float32)
    Wp = np.asarray(Wp, np.float32)
    bp = np.asarray(bp, np.float32)
    if not (np.all(np.asarray(qn_w) == 1) and np.all(np.asarray(qn_b) == 0)
            and np.all(np.asarray(kn_w) == 1) and np.all(np.asarray(kn_b) == 0)):
        raise NotImplementedError("kernel specialized to identity q/k norms")

    nc, in_maps, idxs = prepare(q_x, kv_x, attn_mask, Wq, Wkv, Wp)
    res = bass_utils.run_bass_kernel_spmd(
        nc, in_maps, core_ids=list(range(NCORES)))
    if _profile is not None:
        _profile.append(res)

    # masked-query rows: softmax over an all -1e9 row is uniform, so the
    # output is exactly mean_k(v) @ Wp.T + bp -- pure host math.
    vmean = kv_x.mean(axis=1) @ Wkv[C:].T          # [B, C]
    ymask = vmean @ Wp.T + bp                      # [B, C]
    out = np.empty((B, N, C), np.float32)
    for b in range(B):
        acc = (res.results[G * b]["outT"].astype(np.float32)
               + res.results[G * b + 1]["outT"].astype(np.float32))
        out[b] = ymask[b]
        nb = len(idxs[b])
        out[b, idxs[b]] = acc.T[:nb] + bp
    return out


# revision 10
# speedup vs baseline: 1.7967x; 1.0559x over previous
"""Cross-attention (B=4, N=2048, C=768, H=12, HD=64) on 8 TRN2 NeuronCores.

Sharding: core = (batch, head_group), 4 batches x 2 groups of 6 heads.
Each core computes its group's Q/K/V projections, per-head-dim LayerNorm,
attention, and a partial output projection; the host sums the two group
partials per batch and adds the bias.

Key optimizations over the f32r baseline:
 - The attn_mask masks whole QUERY rows; a masked query's output is exactly
   mean(v) @ Wp + bp, computed on the host in numpy.  The device only sees
   the gathered unmasked queries (NQ ~= 1060 of 2048), cutting S/PV/exp/
   q-proj/out-proj work by ~2x combined with the next point.
 - All matmuls run in fp16: 1 column/cycle streaming (fp32 runs HIGH/LOW
   two-pass) and FastWeightLoad hides LDWEIGHTS (disabled for fp32).
 - S = k~.T @ q~ contracts over HD=64 only, so head pairs are packed into
   the 128-row PE array with tile_position row tiling (auto-derived from
   base_partition 0/64) and run concurrently: S cost halves.
 - LN mean/meansq matmuls are column-tiled (out partitions 0-5 / 32-37 of
   one PSUM tile), the rs/murs broadcast matmuls are row-tiled (partitions
   0-5 / 32-37) - each pair runs concurrently.
 - Softmax needs no row max: LN bounds |S| <= 8, so exp(S) in [3e-4, 3e3],
   safely inside fp16/fp32 range.  The denominator rides along as a ones
   column appended to v (PV out row 64).  O is scaled by 2^-6 before the
   fp16 copy to dodge overflow; the reciprocal uses the same scaled den.
PSUM budget (8 banks): spA+spB (2+2) + poA+poB (1+1) + mix pp (2).
"""

import math

import numpy as np

import concourse.bass as bass
import concourse.mybir as mybir
from concourse import tile
from concourse import bass_utils
from concourse.tile_scheduler import N_PROCS
from concourse.vector_clock import ScopedClock, VectorClock

F32 = mybir.dt.float32
F16 = mybir.dt.float16
AF = mybir.ActivationFunctionType
OP = mybir.AluOpType

B, N, C, H, HD = 4, 2048, 768, 12, 64
G = 2                 # head groups (tensor parallel)
HPG = H // G          # 6 heads per group
CL = HPG * HD         # 384 local channels
P = 128
NT = CL // P          # 3 tiles of local channels
CT = C // P           # 6 contraction tiles
TT = N // P           # 16 key-token tiles
KCH = 512             # kv chunk size
EPS = 1e-5
SCALE = HD ** -0.5
LNB = -0.5 * math.log(HD)   # ln(SCALE): folded into the q-LN exp bias
OSHIFT = 2.0 ** -6          # pre-normalize scale to keep |o| in fp16 range
NCORES = 8

_nop_ctr = [0]


class _FixedTileContext(tile.TileContext):
    """Workaround for a walrus build that allows at most ONE sync-wait per
    instruction: split multi-wait instructions into single-wait NoOps on the
    same engine, and emit the kernel-tail drain's waits as a nop chain."""

    def _split_multiwait(self, insts):
        out = []
        for inst in insts:
            si = getattr(inst, "sync_info", None)
            waits = list(si.on_wait) if si is not None and si.on_wait else []
            if len(waits) > 1:
                eng = inst.engine
                for w in waits[:-1]:
                    _nop_ctr[0] += 1
                    nop = mybir.InstNoOp(
                        name=f"I-waitsplit-{_nop_ctr[0]}", ins=[], outs=[]
                    )
                    nop.engine = eng
                    nop.sync_info = mybir.SyncInfo(on_wait=[w], on_update=[])
                    self.nc.register_instruction(nop)
                    out.append(nop)
                inst.sync_info = mybir.SyncInfo(
                    on_wait=[waits[-1]], on_update=list(si.on_update)
                )
            out.append(inst)
        return out

    def _lower_ordered_insts(self, ordered):
        ordered = {bb: self._split_multiwait(ins) for bb, ins in ordered.items()}
        super()._lower_ordered_insts(ordered)

    def _drain_and_barrier(self, tick_clock, wait_clock):
        gc = tick_clock.global_clock
        vals = [gc[p] for p in range(N_PROCS)]
        for p in [q for q, v in enumerate(vals) if v > 0]:
            partial = VectorClock(
                [vals[q] if q == p else 0 for q in range(N_PROCS)]
            )
            nop = self.nc.sync.nop(nofuse=True, hint="tail_drain_wait")
            wait_clock.add_sem_waits(nop.ins, ScopedClock({None: partial}))
        self.nc.sync.drain()
        self.nc.all_engine_barrier()
        assert self.sems is not None
        popped = self.nc._tile_sem_poison_stack.pop()
        assert popped is self._sem_poison
        self.nc.clear_and_free_semaphores(list(self.sems.allocated().values()))
        self.nc.all_engine_barrier()


def _mm(nc, out, lhsT, rhs, start, stop):
    nc.tensor.matmul(
        out, lhsT, rhs, start=start, stop=stop, skip_group_check=True
    )


def _chunks(total, size=512):
    out, o = [], 0
    while o < total:
        s = min(size, total - o)
        out.append((o, s))
        o += s
    return out


def _step(fill):
    """Advance the round-robin fill queue by one piece (if any)."""
    while fill:
        gen = fill[0]
        try:
            next(gen)
            fill.rotate(-1)
            return
        except StopIteration:
            fill.popleft()


def _drain(gen):
    for _ in gen:
        pass


def _body(tc, aps, NQ):
    import collections
    nc = tc.nc
    qxT, kvxT, wq, wk, wv, wp, colsel, bcast, vones, ones16, outT = aps
    qch = _chunks(NQ)
    kch = _chunks(N)

    cpool = tc.alloc_tile_pool(name="consts", bufs=1)
    bpool = tc.alloc_tile_pool(name="big", bufs=1)

    colsel_sb = cpool.tile([P, NT, HPG], F16, name="colsel", tag="colsel")
    nc.sync.dma_start(colsel_sb[:], colsel[:])
    bcast_sb = cpool.tile([38, NT, P], F16, name="bcast", tag="bcast")
    nc.sync.dma_start(bcast_sb[:], bcast[:])
    ones16_sb = cpool.tile([1, HD], F16, name="ones16", tag="ones16")
    nc.sync.dma_start(ones16_sb[:], ones16[:])
    eps_sb = cpool.tile([HPG, 1], F32, name="eps", tag="eps")
    nc.vector.memset(eps_sb[:], EPS)
    lnb_sb = cpool.tile([HPG, 1], F32, name="lnb", tag="lnb")
    nc.vector.memset(lnb_sb[:], LNB)

    q_sb = [bpool.tile([P, NQ], F16, name=f"q{t}", tag=f"q{t}") for t in range(NT)]
    k_sb = [bpool.tile([P, N], F16, name=f"k{t}", tag=f"k{t}") for t in range(NT)]
    v_sb = bpool.tile([P, TT, HPG, HD + 1], F16, name="v", tag="v")

    # DMA order matters for the cold start: wk first so the first k-proj
    # matmul can issue ASAP; wp (only needed at the tails) last.
    wk_sb = bpool.tile([P, CT, CL], F16, name="wk", tag="wk")
    nc.sync.dma_start(wk_sb[:], wk.rearrange("(ct p) m -> p ct m", p=P))
    wv_sb = bpool.tile([P, CT, CL], F16, name="wv", tag="wv")
    nc.sync.dma_start(wv_sb[:], wv.rearrange("(ct p) m -> p ct m", p=P))
    wq_sb = bpool.tile([P, CT, CL], F16, name="wq", tag="wq")
    nc.sync.dma_start(wq_sb[:], wq.rearrange("(ct p) m -> p ct m", p=P))
    nc.sync.dma_start(v_sb[:, :, :, HD], vones[:])
    wp_sb = bpool.tile([P, NT, C], F16, name="wp", tag="wp")

    # PSUM pools: spA(2) + spB(2) + poA(1) + poB(1) + pp(2x1) = 8 banks
    ps_sp = tc.alloc_tile_pool(name="ps_sp", bufs=1, space="PSUM")
    ps_po = tc.alloc_tile_pool(name="ps_po", bufs=1, space="PSUM")
    ps_mix = tc.alloc_tile_pool(name="ps_mix", bufs=2, space="PSUM")

    x_pool = tc.alloc_tile_pool(name="x", bufs=6)
    sq_pool = tc.alloc_tile_pool(name="sq", bufs=2)
    st_pool = tc.alloc_tile_pool(name="st", bufs=3)
    e_pool = tc.alloc_tile_pool(name="e", bufs=2)
    o_pool = tc.alloc_tile_pool(name="o", bufs=2)
    den_pool = tc.alloc_tile_pool(name="den", bufs=2)
    out_pool = tc.alloc_tile_pool(name="ot", bufs=3)

    nq = len(qch)
    NKB = len(kch)                       # 4 k-blocks of 4 k-tiles each
    KPB = KCH // P                       # 4 k-tiles per block
    # flash accumulators: [65, ch] fp32 per (q-chunk, tile, head)
    o_acc = {}
    for qi, (qo, ch) in enumerate(qch):
        for t in range(NT):
            for hh in range(2):
                o_acc[(qi, t, hh)] = bpool.tile(
                    [HD + 1, ch], F32, name=f"oa{qi}_{t}_{hh}",
                    tag=f"oa{qi}_{t}_{hh}")

    def ln_gen(dst, cs, ch, is_q):
        """Projection LN tail: mean/meansq (column-tiled), rs/murs, apply
        (row-tiled broadcast matmuls).  All PSUM via the 'pp' tag."""
        mums = ps_mix.tile([38, ch], F32, name="mums", tag="pp")
        for t in range(NT):
            sq = sq_pool.tile([P, ch], F16, name="sq", tag="sq")
            nc.vector.tensor_tensor(sq[:], dst[t][:, cs], dst[t][:, cs],
                                    OP.mult)
            _mm(nc, mums[0:HPG, :], colsel_sb[:, t, :], dst[t][:, cs],
                t == 0, t == NT - 1)
            _mm(nc, mums[32:38, :], colsel_sb[:, t, :], sq[:],
                t == 0, t == NT - 1)
        yield
        stf = st_pool.tile([HPG, 3 * ch], F32, name="stf", tag="stf")
        sth = st_pool.tile([38, ch], F16, name="sth", tag="sth")
        mu = stf[:, 0:ch]
        work = stf[:, ch:2 * ch]
        lnv = stf[:, 2 * ch:3 * ch]
        nc.vector.tensor_copy(mu, mums[0:HPG, :])
        nc.vector.scalar_tensor_tensor(work, mu, 1.0, mu, OP.mult, OP.mult)
        nc.vector.tensor_tensor(work, mums[32:38, :], work, OP.subtract)
        nc.scalar.activation(lnv, work, AF.Ln, bias=eps_sb[:])
        # rs = exp(-0.5*ln(var+eps) [+ ln(scale) for q]) ; murs = -mu*rs
        nc.scalar.activation(sth[0:HPG, :], lnv, AF.Exp, scale=-0.5,
                             bias=(lnb_sb[:] if is_q else 0.0))
        nc.vector.scalar_tensor_tensor(sth[32:38, :], mu, -1.0,
                                       sth[0:HPG, :], OP.mult, OP.mult)
        yield
        for t in range(NT):
            rr = ps_mix.tile([P, ch], F32, name="rr", tag="pp")
            _mm(nc, rr[:], bcast_sb[0:HPG, t, :], sth[0:HPG, :], True, True)
            mr = ps_mix.tile([P, ch], F32, name="mr", tag="pp")
            _mm(nc, mr[:], bcast_sb[32:38, t, :], sth[32:38, :], True, True)
            nc.vector.tensor_tensor(dst[t][:, cs], dst[t][:, cs], rr[:],
                                    OP.mult)
            nc.vector.tensor_tensor(dst[t][:, cs], dst[t][:, cs], mr[:],
                                    OP.add)
            yield

    def kv_gen(c):
        co, ch = kch[c]
        cs = slice(co, co + ch)
        xts = []
        for ct in range(CT):
            xt = x_pool.tile([P, ch], F16, name="xt", tag=f"xk{c}")
            nc.sync.dma_start(xt[:], kvxT[ct * P:(ct + 1) * P, cs])
            xts.append(xt)
        yield
        for t in range(NT):
            pp = ps_mix.tile([P, ch], F32, name="pp", tag="pp")
            for ct in range(CT):
                _mm(nc, pp[:], wk_sb[:, ct, t * P:(t + 1) * P], xts[ct][:],
                    ct == 0, ct == CT - 1)
            nc.vector.tensor_copy(k_sb[t][:, cs], pp[:])
            yield
        for tl in range(ch // P):
            ttk = co // P + tl
            vp = ps_mix.tile([P, CL], F32, name="vp", tag="pp")
            for ct in range(CT):
                _mm(nc, vp[:], xts[ct][:, tl * P:(tl + 1) * P],
                    wv_sb[:, ct, :], ct == 0, ct == CT - 1)
            nc.vector.tensor_copy(
                v_sb[:, ttk, :, 0:HD],
                vp[:].rearrange("p (h d) -> p h d", h=HPG))
            yield
        yield from ln_gen(k_sb, cs, ch, False)

    def q_gen(i):
        qo, ch = qch[i]
        cs = slice(qo, qo + ch)
        xts = []
        for ct in range(CT):
            xt = x_pool.tile([P, ch], F16, name="xt", tag=f"xq{i}")
            nc.sync.dma_start(xt[:], qxT[ct * P:(ct + 1) * P, cs])
            xts.append(xt)
        yield
        for t in range(NT):
            pp = ps_mix.tile([P, ch], F32, name="pp", tag="pp")
            for ct in range(CT):
                _mm(nc, pp[:], wq_sb[:, ct, t * P:(t + 1) * P], xts[ct][:],
                    ct == 0, ct == CT - 1)
            nc.vector.tensor_copy(q_sb[t][:, cs], pp[:])
            yield
        yield from ln_gen(q_sb, cs, ch, True)

    def attn_unit(qi, t, blk, fill):
        """S + exp + PV for one (q-chunk, head-pair) over one k-block
        (4 k-tiles), accumulating into o_acc via DVE."""
        qo, ch = qch[qi]
        qs = slice(qo, qo + ch)
        hA, hB = 2 * t, 2 * t + 1
        poA = ps_po.tile([HD + 1, ch], F32, name="poA", tag="poA")
        poB = ps_po.tile([HD + 1, ch], F32, name="poB", tag="poB")
        for g in range(KPB // 2):
            spA = ps_sp.tile([P, 2 * ch], F32, name="spA", tag="spA")
            spB = ps_sp.tile([P, 2 * ch], F32, name="spB", tag="spB")
            for j in range(2):
                kt = blk * KPB + 2 * g + j
                # row-tiled pair: head A in PE rows 0-63, head B in 64-127
                _mm(nc, spA[:, j * ch:(j + 1) * ch],
                    k_sb[t][0:HD, kt * P:(kt + 1) * P],
                    q_sb[t][0:HD, qs], True, True)
                _mm(nc, spB[:, j * ch:(j + 1) * ch],
                    k_sb[t][HD:P, kt * P:(kt + 1) * P],
                    q_sb[t][HD:P, qs], True, True)
            eA = e_pool.tile([P, 2 * ch], F16, name="eA", tag="eA")
            eB = e_pool.tile([P, 2 * ch], F16, name="eB", tag="eB")
            nc.scalar.activation(eA[:], spA[:], AF.Exp)
            nc.scalar.activation(eB[:], spB[:], AF.Exp)
            for j in range(2):
                lk = 2 * g + j
                kt = blk * KPB + lk
                _mm(nc, poA[:], v_sb[:, kt, hA, :],
                    eA[:, j * ch:(j + 1) * ch], lk == 0, lk == KPB - 1)
                _mm(nc, poB[:], v_sb[:, kt, hB, :],
                    eB[:, j * ch:(j + 1) * ch], lk == 0, lk == KPB - 1)
            _step(fill)
        accA, accB = o_acc[(qi, t, 0)], o_acc[(qi, t, 1)]
        if blk == 0:
            nc.vector.tensor_copy(accA[:], poA[:])
            nc.vector.tensor_copy(accB[:], poB[:])
        else:
            nc.vector.tensor_tensor(accA[:], accA[:], poA[:], OP.add)
            nc.vector.tensor_tensor(accB[:], accB[:], poB[:], OP.add)

    def tail_gen(qi):
        """Normalize by the softmax denominator and project out."""
        qo, ch = qch[qi]
        qs = slice(qo, qo + ch)
        den = den_pool.tile([1, HPG * ch], F32, name="den", tag="den")
        for t in range(NT):
            for hh in range(2):
                h = 2 * t + hh
                nc.vector.tensor_scalar_mul(
                    den[0:1, h * ch:(h + 1) * ch],
                    o_acc[(qi, t, hh)][HD:HD + 1, :], OSHIFT)
        # batched reciprocal: repack [1, 6*ch] -> [32, 6*ch/32] (DVE
        # reciprocal cost scales with free size only), invert, unpack fp16
        w32 = HPG * ch // 32
        dpk = den_pool.tile([32, w32], F32, name="dpk", tag="dpk")
        nc.sync.dma_start(dpk[:], den[0:1, :])
        rpk = den_pool.tile([32, w32], F32, name="rpk", tag="rpk")
        nc.vector.reciprocal(rpk[:], dpk[:])
        rpk16 = den_pool.tile([32, w32], F16, name="rpk16", tag="rpk16")
        nc.vector.tensor_copy(rpk16[:], rpk[:])
        denr = den_pool.tile([1, HPG * ch], F16, name="denr", tag="denr")
        nc.sync.dma_start(denr[0:1, :], rpk16[:])
        yield
        o_t = [o_pool.tile([P, ch], F16, name=f"o{t}", tag=f"o{t}")
               for t in range(NT)]
        for t in range(NT):
            for hh in range(2):
                h = 2 * t + hh
                rb = ps_mix.tile([HD, ch], F32, name="rb", tag="pp")
                _mm(nc, rb[:], ones16_sb[:],
                    denr[0:1, h * ch:(h + 1) * ch], True, True)
                # o = (acc * 2^-6) * (1 / (den * 2^-6)) -- fp16-safe
                nc.vector.scalar_tensor_tensor(
                    o_t[t][hh * HD:(hh + 1) * HD, :],
                    o_acc[(qi, t, hh)][0:HD, :], OSHIFT, rb[:],
                    OP.mult, OP.mult)
            yield
        for m in range(CT):
            pp = ps_mix.tile([P, ch], F32, name="op", tag="pp")
            for t in range(NT):
                _mm(nc, pp[:], wp_sb[:, t, m * P:(m + 1) * P], o_t[t][:],
                    t == 0, t == NT - 1)
            ot = out_pool.tile([P, ch], F16, name="ot", tag="ot")
            nc.vector.tensor_copy(ot[:], pp[:])
            nc.sync.dma_start(outT[m * P:(m + 1) * P, qs], ot[:])
            yield

    # ---- schedule ----------------------------------------------------
    # Head: kv blocks 0-1 and q chunk 0 run dense (PE-bound, ACT idle).
    # Attention (exp/ACT-bound) then runs block-major with the remaining
    # projection work fed into the PE's idle slots so the HAM clock gate
    # never sees an idle PE window: block0 <- q chunks 1+, block1 <- kv2,
    # block2 <- kv3, block3 <- per-chunk output tails.
    # Head: kv0/kv1/q0 interleaved round-robin so one chunk's serial LN
    # chain (DVE+ACT) overlaps another's projection matmuls -- a >3.4us
    # PE-idle window here demotes the HAM clock gate to 1.2 GHz.
    kvg = {2: kv_gen(2), 3: kv_gen(3)}
    qgens = {i: q_gen(i) for i in range(1, nq)}
    head = collections.deque([kv_gen(0), kv_gen(1), q_gen(0)])
    for _ in range(3):
        _step(head)            # the three x-DMA bursts
    for gen in list(kvg.values()) + list(qgens.values()):
        next(gen, None)        # prefetch x-DMAs for later chunks too
    nc.sync.dma_start(wp_sb[:], wp.rearrange("(t p) m -> p t m", p=P))
    while head:
        _step(head)
    fill = collections.deque(qgens.values())
    for blk in range(NKB):
        # producers must be fully issued before their consumers (the tile
        # framework orders by issue): force-drain whatever the fill queue
        # hasn't finished by the time it's needed.
        if blk == 1 and 2 in kvg:
            fill.append(kvg[2])
        elif blk == 2:
            _drain(kvg[2])
            fill.append(kvg[3])
        elif blk == 3:
            _drain(kvg[3])
        for qi in range(nq):
            if blk == 0 and qi in qgens:
                _drain(qgens[qi])
            for t in range(NT):
                attn_unit(qi, t, blk, fill)
            if blk == NKB - 1:
                fill.append(tail_gen(qi))
    while fill:
        _step(fill)

    for pool in (out_pool, den_pool, o_pool, e_pool, st_pool, sq_pool,
                 x_pool, ps_mix, ps_po, ps_sp, bpool, cpool):
        pool.release()


def build_bass(NQ):
    nc = bass.Bass(trn_type="TRN2", debug=False, num_devices=NCORES)
    qxT = nc.dram_tensor("qxT", [C, NQ], F16, kind="ExternalInput").ap()
    kvxT = nc.dram_tensor("kvxT", [C, N], F16, kind="ExternalInput").ap()
    wq = nc.dram_tensor("wq", [C, CL], F16, kind="ExternalInput").ap()
    wk = nc.dram_tensor("wk", [C, CL], F16, kind="ExternalInput").ap()
    wv = nc.dram_tensor("wv", [C, CL], F16, kind="ExternalInput").ap()
    wp = nc.dram_tensor("wp", [CL, C], F16, kind="ExternalInput").ap()
    colsel = nc.dram_tensor("colsel", [P, NT, HPG], F16,
                            kind="ExternalInput").ap()
    bcast = nc.dram_tensor("bcast", [38, NT, P], F16,
                           kind="ExternalInput").ap()
    vones = nc.dram_tensor("vones", [P, TT, HPG], F16,
                           kind="ExternalInput").ap()
    ones16 = nc.dram_tensor("ones16", [1, HD], F16,
                            kind="ExternalInput").ap()
    outT = nc.dram_tensor("outT", [C, NQ], F16, kind="ExternalOutput").ap()
    aps = (qxT, kvxT, wq, wk, wv, wp, colsel, bcast, vones, ones16, outT)
    with _FixedTileContext(nc) as tc:
        _body(tc, aps, NQ)
    return nc


def make_in_maps(q_x, kv_x, attn_mask, Wq, Wkv, Wp, NQ, idxs):
    colsel = np.zeros((P, NT, HPG), np.float16)
    bcast = np.zeros((38, NT, P), np.float16)
    for t in range(NT):
        for pp in range(P):
            h = 2 * t + pp // HD
            colsel[pp, t, h] = 1.0 / HD
            bcast[h, t, pp] = 1.0
    bcast[32:38] = bcast[0:HPG]  # mirror for the row-tiled murs broadcast
    ones16 = np.ones((1, HD), np.float16)
    vones = np.ones((P, TT, HPG), np.float16)

    in_maps = []
    for core in range(NCORES):
        b, g = core // G, core % G
        sl = slice(g * CL, (g + 1) * CL)
        idx = idxs[b]
        pad = np.zeros(NQ, np.int64)
        pad[:len(idx)] = idx
        if len(idx) < NQ:
            pad[len(idx):] = idx[0] if len(idx) else 0
        in_maps.append({
            "qxT": np.ascontiguousarray(q_x[b][pad].T.astype(np.float16)),
            "kvxT": np.ascontiguousarray(kv_x[b].T.astype(np.float16)),
            "wq": np.ascontiguousarray(Wq[sl].T.astype(np.float16)),
            "wk": np.ascontiguousarray(Wkv[sl].T.astype(np.float16)),
            "wv": np.ascontiguousarray(
                Wkv[C + g * CL:C + (g + 1) * CL].T.astype(np.float16)),
            "wp": np.ascontiguousarray(Wp[:, sl].T.astype(np.float16)),
            "colsel": colsel,
            "bcast": bcast,
            "vones": vones,
            "ones16": ones16,
        })
    return in_maps


_NC_CACHE = {}


def get_nc(NQ):
    if NQ not in _NC_CACHE:
        _NC_CACHE[NQ] = build_bass(NQ)
    return _NC_CACHE[NQ]


def prepare(q_x, kv_x, attn_mask, Wq, Wkv, Wp):
    mask = np.asarray(attn_mask).astype(bool)
    idxs = [np.flatnonzero(mask[b]) for b in range(B)]
    numax = max(1, max(len(i) for i in idxs))
    NQ = ((numax + 31) // 32) * 32
    nc = get_nc(NQ)
    in_maps = make_in_maps(q_x, kv_x, mask, Wq, Wkv, Wp, NQ, idxs)
    return nc, in_maps, idxs


def kernel(q_x, kv_x, attn_mask, Wq, Wkv, qn_w, qn_b, kn_w, kn_b, Wp, bp,
           _profile=None):
    q_x = np.asarray(q_x, np.float32)
    kv_x = np.asarray(kv_x, np.float32)
    Wq = np.asarray(Wq, np.float32)
    Wkv = np.asarray(Wkv, np.float32)
    Wp = np.asarray(Wp, np.float32)
    bp = np.asarray(bp, np.float32)
    if not (np.all(np.asarray(qn_w) == 1) and np.all(np.asarray(qn_b) == 0)
            and np.all(np.asarray(kn_w) == 1) and np.all(np.asarray(kn_b) == 0)):
        raise NotImplementedError("kernel specialized to identity q/k norms")

    nc, in_maps, idxs = prepare(q_x, kv_x, attn_mask, Wq, Wkv, Wp)
    res = bass_utils.run_bass_kernel_spmd(
        nc, in_maps, core_ids=list(range(NCORES)))
    if _profile is not None:
        _profile.append(res)

    # masked-query rows: softmax over an all -1e9 row is uniform, so the
    # output is exactly mean_k(v) @ Wp.T + bp -- pure host math.
    vmean = kv_x.mean(axis=1) @ Wkv[C:].T          # [B, C]
    ymask = vmean @ Wp.T + bp                      # [B, C]
    out = np.empty((B, N, C), np.float32)
    for b in range(B):
        acc = (res.results[G * b]["outT"].astype(np.float32)
               + res.results[G * b + 1]["outT"].astype(np.float32))
        out[b] = ymask[b]
        nb = len(idxs[b])
        out[b, idxs[b]] = acc.T[:nb] + bp
    return out


# revision 11
# speedup vs baseline: 1.8413x; 1.0248x over previous
"""Cross-attention (B=4, N=2048, C=768, H=12, HD=64) on 8 TRN2 NeuronCores.

Sharding: core = (batch, head_group), 4 batches x 2 groups of 6 heads.
Each core computes its group's Q/K/V projections, per-head-dim LayerNorm,
attention, and a partial output projection; the host sums the two group
partials per batch and adds the bias.

Key optimizations over the f32r baseline:
 - The attn_mask masks whole QUERY rows; a masked query's output is exactly
   mean(v) @ Wp + bp, computed on the host in numpy.  The device only sees
   the gathered unmasked queries (NQ ~= 1060 of 2048), cutting S/PV/exp/
   q-proj/out-proj work by ~2x combined with the next point.
 - All matmuls run in fp16: 1 column/cycle streaming (fp32 runs HIGH/LOW
   two-pass) and FastWeightLoad hides LDWEIGHTS (disabled for fp32).
 - S = k~.T @ q~ contracts over HD=64 only, so head pairs are packed into
   the 128-row PE array with tile_position row tiling (auto-derived from
   base_partition 0/64) and run concurrently: S cost halves.
 - LN mean/meansq matmuls are column-tiled (out partitions 0-5 / 32-37 of
   one PSUM tile), the rs/murs broadcast matmuls are row-tiled (partitions
   0-5 / 32-37) - each pair runs concurrently.
 - Softmax needs no row max: LN bounds |S| <= 8, so exp(S) in [3e-4, 3e3],
   safely inside fp16/fp32 range.  The denominator rides along as a ones
   column appended to v (PV out row 64).  O is scaled by 2^-6 before the
   fp16 copy to dodge overflow; the reciprocal uses the same scaled den.
PSUM budget (8 banks): spA+spB (2+2) + poA+poB (1+1) + mix pp (2).
"""

import math

import numpy as np

import concourse.bass as bass
import concourse.mybir as mybir
from concourse import tile
from concourse import bass_utils
from concourse.tile_scheduler import N_PROCS
from concourse.vector_clock import ScopedClock, VectorClock

F32 = mybir.dt.float32
F16 = mybir.dt.float16
AF = mybir.ActivationFunctionType
OP = mybir.AluOpType

B, N, C, H, HD = 4, 2048, 768, 12, 64
G = 2                 # head groups (tensor parallel)
HPG = H // G          # 6 heads per group
CL = HPG * HD         # 384 local channels
P = 128
NT = CL // P          # 3 tiles of local channels
CT = C // P           # 6 contraction tiles
TT = N // P           # 16 key-token tiles
KCH = 512             # kv chunk size
EPS = 1e-5
SCALE = HD ** -0.5
LNB = -0.5 * math.log(HD)   # ln(SCALE): folded into the q-LN exp bias
OSHIFT = 2.0 ** -6          # pre-normalize scale to keep |o| in fp16 range
NCORES = 8

_nop_ctr = [0]


class _FixedTileContext(tile.TileContext):
    """Workaround for a walrus build that allows at most ONE sync-wait per
    instruction: split multi-wait instructions into single-wait NoOps on the
    same engine, and emit the kernel-tail drain's waits as a nop chain."""

    def _split_multiwait(self, insts):
        out = []
        for inst in insts:
            si = getattr(inst, "sync_info", None)
            waits = list(si.on_wait) if si is not None and si.on_wait else []
            if len(waits) > 1:
                eng = inst.engine
                for w in waits[:-1]:
                    _nop_ctr[0] += 1
                    nop = mybir.InstNoOp(
                        name=f"I-waitsplit-{_nop_ctr[0]}", ins=[], outs=[]
                    )
                    nop.engine = eng
                    nop.sync_info = mybir.SyncInfo(on_wait=[w], on_update=[])
                    self.nc.register_instruction(nop)
                    out.append(nop)
                inst.sync_info = mybir.SyncInfo(
                    on_wait=[waits[-1]], on_update=list(si.on_update)
                )
            out.append(inst)
        return out

    def _lower_ordered_insts(self, ordered):
        ordered = {bb: self._split_multiwait(ins) for bb, ins in ordered.items()}
        super()._lower_ordered_insts(ordered)

    def _drain_and_barrier(self, tick_clock, wait_clock):
        gc = tick_clock.global_clock
        vals = [gc[p] for p in range(N_PROCS)]
        for p in [q for q, v in enumerate(vals) if v > 0]:
            partial = VectorClock(
                [vals[q] if q == p else 0 for q in range(N_PROCS)]
            )
            nop = self.nc.sync.nop(nofuse=True, hint="tail_drain_wait")
            wait_clock.add_sem_waits(nop.ins, ScopedClock({None: partial}))
        self.nc.sync.drain()
        self.nc.all_engine_barrier()
        assert self.sems is not None
        popped = self.nc._tile_sem_poison_stack.pop()
        assert popped is self._sem_poison
        self.nc.clear_and_free_semaphores(list(self.sems.allocated().values()))
        self.nc.all_engine_barrier()


def _mm(nc, out, lhsT, rhs, start, stop):
    nc.tensor.matmul(
        out, lhsT, rhs, start=start, stop=stop, skip_group_check=True
    )


def _chunks(total, size=512):
    out, o = [], 0
    while o < total:
        s = min(size, total - o)
        out.append((o, s))
        o += s
    return out


def _step(fill):
    """Advance the round-robin fill queue by one piece (if any)."""
    while fill:
        gen = fill[0]
        try:
            next(gen)
            fill.rotate(-1)
            return
        except StopIteration:
            fill.popleft()


def _drain(gen):
    for _ in gen:
        pass


def _body(tc, aps, NQ):
    import collections
    nc = tc.nc
    qxT, kvxT, wq, wk, wv, wp, colsel, bcast, vones, ones16, outT = aps
    qch = _chunks(NQ)
    kch = _chunks(N)

    cpool = tc.alloc_tile_pool(name="consts", bufs=1)
    bpool = tc.alloc_tile_pool(name="big", bufs=1)

    colsel_sb = cpool.tile([P, NT, HPG], F16, name="colsel", tag="colsel")
    nc.sync.dma_start(colsel_sb[:], colsel[:])
    bcast_sb = cpool.tile([38, NT, P], F16, name="bcast", tag="bcast")
    nc.sync.dma_start(bcast_sb[:], bcast[:])
    ones16_sb = cpool.tile([1, HD], F16, name="ones16", tag="ones16")
    nc.sync.dma_start(ones16_sb[:], ones16[:])
    eps_sb = cpool.tile([HPG, 1], F32, name="eps", tag="eps")
    nc.vector.memset(eps_sb[:], EPS)
    lnb_sb = cpool.tile([HPG, 1], F32, name="lnb", tag="lnb")
    nc.vector.memset(lnb_sb[:], LNB)

    q_sb = [bpool.tile([P, NQ], F16, name=f"q{t}", tag=f"q{t}") for t in range(NT)]
    k_sb = [bpool.tile([P, N], F16, name=f"k{t}", tag=f"k{t}") for t in range(NT)]
    v_sb = bpool.tile([P, TT, HPG, HD + 1], F16, name="v", tag="v")

    # DMA order matters for the cold start: wk first so the first k-proj
    # matmul can issue ASAP; wp (only needed at the tails) last.
    wk_sb = bpool.tile([P, CT, CL], F16, name="wk", tag="wk")
    nc.sync.dma_start(wk_sb[:], wk.rearrange("(ct p) m -> p ct m", p=P))
    wv_sb = bpool.tile([P, CT, CL], F16, name="wv", tag="wv")
    nc.sync.dma_start(wv_sb[:], wv.rearrange("(ct p) m -> p ct m", p=P))
    wq_sb = bpool.tile([P, CT, CL], F16, name="wq", tag="wq")
    nc.sync.dma_start(wq_sb[:], wq.rearrange("(ct p) m -> p ct m", p=P))
    nc.sync.dma_start(v_sb[:, :, :, HD], vones[:])
    wp_sb = bpool.tile([P, NT, C], F16, name="wp", tag="wp")

    # PSUM pools: spA(2) + spB(2) + poA(1) + poB(1) + pp(2x1) = 8 banks
    ps_sp = tc.alloc_tile_pool(name="ps_sp", bufs=1, space="PSUM")
    ps_po = tc.alloc_tile_pool(name="ps_po", bufs=1, space="PSUM")
    ps_mix = tc.alloc_tile_pool(name="ps_mix", bufs=2, space="PSUM")

    x_pool = tc.alloc_tile_pool(name="x", bufs=6)
    sq_pool = tc.alloc_tile_pool(name="sq", bufs=2)
    st_pool = tc.alloc_tile_pool(name="st", bufs=3)
    e_pool = tc.alloc_tile_pool(name="e", bufs=2)
    o_pool = tc.alloc_tile_pool(name="o", bufs=2)
    den_pool = tc.alloc_tile_pool(name="den", bufs=2)
    out_pool = tc.alloc_tile_pool(name="ot", bufs=3)

    nq = len(qch)
    NKB = len(kch)                       # 4 k-blocks of 4 k-tiles each
    KPB = KCH // P                       # 4 k-tiles per block
    # flash accumulators: [65, ch] fp32 per (q-chunk, tile, head)
    o_acc = {}
    for qi, (qo, ch) in enumerate(qch):
        for t in range(NT):
            for hh in range(2):
                o_acc[(qi, t, hh)] = bpool.tile(
                    [HD + 1, ch], F32, name=f"oa{qi}_{t}_{hh}",
                    tag=f"oa{qi}_{t}_{hh}")

    def ln_gen(dst, cs, ch, is_q):
        """Projection LN tail: mean/meansq (column-tiled), rs/murs, apply
        (row-tiled broadcast matmuls).  All PSUM via the 'pp' tag."""
        mums = ps_mix.tile([38, ch], F32, name="mums", tag="pp")
        for t in range(NT):
            sq = sq_pool.tile([P, ch], F16, name="sq", tag="sq")
            nc.vector.tensor_tensor(sq[:], dst[t][:, cs], dst[t][:, cs],
                                    OP.mult)
            _mm(nc, mums[0:HPG, :], colsel_sb[:, t, :], dst[t][:, cs],
                t == 0, t == NT - 1)
            _mm(nc, mums[32:38, :], colsel_sb[:, t, :], sq[:],
                t == 0, t == NT - 1)
        yield
        stf = st_pool.tile([HPG, 3 * ch], F32, name="stf", tag="stf")
        sth = st_pool.tile([38, ch], F16, name="sth", tag="sth")
        mu = stf[:, 0:ch]
        work = stf[:, ch:2 * ch]
        lnv = stf[:, 2 * ch:3 * ch]
        nc.vector.tensor_copy(mu, mums[0:HPG, :])
        nc.vector.scalar_tensor_tensor(work, mu, 1.0, mu, OP.mult, OP.mult)
        nc.vector.tensor_tensor(work, mums[32:38, :], work, OP.subtract)
        nc.scalar.activation(lnv, work, AF.Ln, bias=eps_sb[:])
        # rs = exp(-0.5*ln(var+eps) [+ ln(scale) for q]) ; murs = -mu*rs
        nc.scalar.activation(sth[0:HPG, :], lnv, AF.Exp, scale=-0.5,
                             bias=(lnb_sb[:] if is_q else 0.0))
        nc.vector.scalar_tensor_tensor(sth[32:38, :], mu, -1.0,
                                       sth[0:HPG, :], OP.mult, OP.mult)
        yield
        for t in range(NT):
            rr = ps_mix.tile([P, ch], F32, name="rr", tag="pp")
            _mm(nc, rr[:], bcast_sb[0:HPG, t, :], sth[0:HPG, :], True, True)
            mr = ps_mix.tile([P, ch], F32, name="mr", tag="pp")
            _mm(nc, mr[:], bcast_sb[32:38, t, :], sth[32:38, :], True, True)
            nc.vector.tensor_tensor(dst[t][:, cs], dst[t][:, cs], rr[:],
                                    OP.mult)
            nc.vector.tensor_tensor(dst[t][:, cs], dst[t][:, cs], mr[:],
                                    OP.add)
            yield

    def kv_gen(c):
        co, ch = kch[c]
        cs = slice(co, co + ch)
        xts = []
        for ct in range(CT):
            xt = x_pool.tile([P, ch], F16, name="xt", tag=f"xk{c}")
            nc.sync.dma_start(xt[:], kvxT[ct * P:(ct + 1) * P, cs])
            xts.append(xt)
        yield
        for t in range(NT):
            pp = ps_mix.tile([P, ch], F32, name="pp", tag="pp")
            for ct in range(CT):
                _mm(nc, pp[:], wk_sb[:, ct, t * P:(t + 1) * P], xts[ct][:],
                    ct == 0, ct == CT - 1)
            nc.vector.tensor_copy(k_sb[t][:, cs], pp[:])
            yield
        for tl in range(ch // P):
            ttk = co // P + tl
            vp = ps_mix.tile([P, CL], F32, name="vp", tag="pp")
            for ct in range(CT):
                _mm(nc, vp[:], xts[ct][:, tl * P:(tl + 1) * P],
                    wv_sb[:, ct, :], ct == 0, ct == CT - 1)
            nc.vector.tensor_copy(
                v_sb[:, ttk, :, 0:HD],
                vp[:].rearrange("p (h d) -> p h d", h=HPG))
            yield
        yield from ln_gen(k_sb, cs, ch, False)

    def q_gen(i):
        qo, ch = qch[i]
        cs = slice(qo, qo + ch)
        xts = []
        for ct in range(CT):
            xt = x_pool.tile([P, ch], F16, name="xt", tag=f"xq{i}")
            nc.sync.dma_start(xt[:], qxT[ct * P:(ct + 1) * P, cs])
            xts.append(xt)
        yield
        for t in range(NT):
            pp = ps_mix.tile([P, ch], F32, name="pp", tag="pp")
            for ct in range(CT):
                _mm(nc, pp[:], wq_sb[:, ct, t * P:(t + 1) * P], xts[ct][:],
                    ct == 0, ct == CT - 1)
            nc.vector.tensor_copy(q_sb[t][:, cs], pp[:])
            yield
        yield from ln_gen(q_sb, cs, ch, True)

    def attn_unit(qi, t, blk, fill):
        """S + exp + PV for one (q-chunk, head-pair) over one k-block
        (4 k-tiles), accumulating into o_acc via DVE."""
        qo, ch = qch[qi]
        qs = slice(qo, qo + ch)
        hA, hB = 2 * t, 2 * t + 1
        poA = ps_po.tile([HD + 1, ch], F32, name="poA", tag="poA")
        poB = ps_po.tile([HD + 1, ch], F32, name="poB", tag="poB")
        for g in range(KPB // 2):
            spA = ps_sp.tile([P, 2 * ch], F32, name="spA", tag="spA")
            spB = ps_sp.tile([P, 2 * ch], F32, name="spB", tag="spB")
            for j in range(2):
                kt = blk * KPB + 2 * g + j
                # row-tiled pair: head A in PE rows 0-63, head B in 64-127
                _mm(nc, spA[:, j * ch:(j + 1) * ch],
                    k_sb[t][0:HD, kt * P:(kt + 1) * P],
                    q_sb[t][0:HD, qs], True, True)
                _mm(nc, spB[:, j * ch:(j + 1) * ch],
                    k_sb[t][HD:P, kt * P:(kt + 1) * P],
                    q_sb[t][HD:P, qs], True, True)
            eA = e_pool.tile([P, 2 * ch], F16, name="eA", tag="eA")
            eB = e_pool.tile([P, 2 * ch], F16, name="eB", tag="eB")
            nc.scalar.activation(eA[:], spA[:], AF.Exp)
            nc.scalar.activation(eB[:], spB[:], AF.Exp)
            for j in range(2):
                lk = 2 * g + j
                kt = blk * KPB + lk
                _mm(nc, poA[:], v_sb[:, kt, hA, :],
                    eA[:, j * ch:(j + 1) * ch], lk == 0, lk == KPB - 1)
                _mm(nc, poB[:], v_sb[:, kt, hB, :],
                    eB[:, j * ch:(j + 1) * ch], lk == 0, lk == KPB - 1)
            _step(fill)
        accA, accB = o_acc[(qi, t, 0)], o_acc[(qi, t, 1)]
        if blk == 0:
            nc.vector.tensor_copy(accA[:], poA[:])
            nc.vector.tensor_copy(accB[:], poB[:])
        else:
            nc.vector.tensor_tensor(accA[:], accA[:], poA[:], OP.add)
            nc.vector.tensor_tensor(accB[:], accB[:], poB[:], OP.add)

    def tail_gen(qi):
        """Normalize by the softmax denominator and project out."""
        qo, ch = qch[qi]
        qs = slice(qo, qo + ch)
        den = den_pool.tile([1, HPG * ch], F32, name="den", tag="den")
        for t in range(NT):
            for hh in range(2):
                h = 2 * t + hh
                nc.vector.tensor_scalar_mul(
                    den[0:1, h * ch:(h + 1) * ch],
                    o_acc[(qi, t, hh)][HD:HD + 1, :], OSHIFT)
        # batched reciprocal: repack [1, 6*ch] -> [32, 6*ch/32] (DVE
        # reciprocal cost scales with free size only), invert, unpack fp16
        w32 = HPG * ch // 32
        dpk = den_pool.tile([32, w32], F32, name="dpk", tag="dpk")
        nc.sync.dma_start(dpk[:], den[0:1, :])
        rpk = den_pool.tile([32, w32], F32, name="rpk", tag="rpk")
        nc.vector.reciprocal(rpk[:], dpk[:])
        rpk16 = den_pool.tile([32, w32], F16, name="rpk16", tag="rpk16")
        nc.vector.tensor_copy(rpk16[:], rpk[:])
        denr = den_pool.tile([1, HPG * ch], F16, name="denr", tag="denr")
        nc.sync.dma_start(denr[0:1, :], rpk16[:])
        yield
        o_t = [o_pool.tile([P, ch], F16, name=f"o{t}", tag=f"o{t}")
               for t in range(NT)]
        for t in range(NT):
            for hh in range(2):
                h = 2 * t + hh
                rb = ps_mix.tile([HD, ch], F32, name="rb", tag="pp")
                _mm(nc, rb[:], ones16_sb[:],
                    denr[0:1, h * ch:(h + 1) * ch], True, True)
                # o = (acc * 2^-6) * (1 / (den * 2^-6)) -- fp16-safe
                nc.vector.scalar_tensor_tensor(
                    o_t[t][hh * HD:(hh + 1) * HD, :],
                    o_acc[(qi, t, hh)][0:HD, :], OSHIFT, rb[:],
                    OP.mult, OP.mult)
            yield
        for m in range(CT):
            pp = ps_mix.tile([P, ch], F32, name="op", tag="pp")
            for t in range(NT):
                _mm(nc, pp[:], wp_sb[:, t, m * P:(m + 1) * P], o_t[t][:],
                    t == 0, t == NT - 1)
            ot = out_pool.tile([P, ch], F16, name="ot", tag="ot")
            nc.vector.tensor_copy(ot[:], pp[:])
            nc.sync.dma_start(outT[m * P:(m + 1) * P, qs], ot[:])
            yield

    # ---- schedule ----------------------------------------------------
    # Head: kv blocks 0-1 and q chunk 0 run dense (PE-bound, ACT idle).
    # Attention (exp/ACT-bound) then runs block-major with the remaining
    # projection work fed into the PE's idle slots so the HAM clock gate
    # never sees an idle PE window: block0 <- q chunks 1+, block1 <- kv2,
    # block2 <- kv3, block3 <- per-chunk output tails.
    # Head: only kv0 + q0, interleaved round-robin so one chunk's serial
    # LN chain (DVE+ACT) overlaps the other's projection matmuls -- a
    # >3.4us PE-idle window demotes the HAM clock gate to 1.2 GHz.  All
    # remaining projection work (kv1-3, q1+) feeds the attention blocks'
    # PE idle slots: attention alone is exp/ACT-bound at ~56% PE busy,
    # and a window below ~85% busy also demotes the clock.
    kvg = {1: kv_gen(1), 2: kv_gen(2), 3: kv_gen(3)}
    qgens = {i: q_gen(i) for i in range(1, nq)}
    head = collections.deque([kv_gen(0), q_gen(0)])
    for _ in range(2):
        _step(head)            # the two x-DMA bursts
    for gen in list(kvg.values()) + list(qgens.values()):
        next(gen, None)        # prefetch x-DMAs for later chunks too
    nc.sync.dma_start(wp_sb[:], wp.rearrange("(t p) m -> p t m", p=P))
    while head:
        _step(head)
    fill = collections.deque(qgens.values())
    fill.append(kvg[1])
    for blk in range(NKB):
        # producers must be fully issued before their consumers (the tile
        # framework orders by issue): force-drain whatever the fill queue
        # hasn't finished by the time it's needed.
        if blk == 1:
            _drain(kvg[1])
            fill.append(kvg[2])
        elif blk == 2:
            _drain(kvg[2])
            fill.append(kvg[3])
        elif blk == 3:
            _drain(kvg[3])
        for qi in range(nq):
            if blk == 0 and qi in qgens:
                _drain(qgens[qi])
            for t in range(NT):
                attn_unit(qi, t, blk, fill)
            if blk == NKB - 1:
                fill.append(tail_gen(qi))
    while fill:
        _step(fill)

    for pool in (out_pool, den_pool, o_pool, e_pool, st_pool, sq_pool,
                 x_pool, ps_mix, ps_po, ps_sp, bpool, cpool):
        pool.release()


def build_bass(NQ):
    nc = bass.Bass(trn_type="TRN2", debug=False, num_devices=NCORES)
    qxT = nc.dram_tensor("qxT", [C, NQ], F16, kind="ExternalInput").ap()
    kvxT = nc.dram_tensor("kvxT", [C, N], F16, kind="ExternalInput").ap()
    wq = nc.dram_tensor("wq", [C, CL], F16, kind="ExternalInput").ap()
    wk = nc.dram_tensor("wk", [C, CL], F16, kind="ExternalInput").ap()
    wv = nc.dram_tensor("wv", [C, CL], F16, kind="ExternalInput").ap()
    wp = nc.dram_tensor("wp", [CL, C], F16, kind="ExternalInput").ap()
    colsel = nc.dram_tensor("colsel", [P, NT, HPG], F16,
                            kind="ExternalInput").ap()
    bcast = nc.dram_tensor("bcast", [38, NT, P], F16,
                           kind="ExternalInput").ap()
    vones = nc.dram_tensor("vones", [P, TT, HPG], F16,
                           kind="ExternalInput").ap()
    ones16 = nc.dram_tensor("ones16", [1, HD], F16,
                            kind="ExternalInput").ap()
    outT = nc.dram_tensor("outT", [C, NQ], F16, kind="ExternalOutput").ap()
    aps = (qxT, kvxT, wq, wk, wv, wp, colsel, bcast, vones, ones16, outT)
    with _FixedTileContext(nc) as tc:
        _body(tc, aps, NQ)
    return nc


def make_in_maps(q_x, kv_x, attn_mask, Wq, Wkv, Wp, NQ, idxs):
    colsel = np.zeros((P, NT, HPG), np.float16)
    bcast = np.zeros((38, NT, P), np.float16)
    for t in range(NT):
        for pp in range(P):
            h = 2 * t + pp // HD
            colsel[pp, t, h] = 1.0 / HD
            bcast[h, t, pp] = 1.0
    bcast[32:38] = bcast[0:HPG]  # mirror for the row-tiled murs broadcast
    ones16 = np.ones((1, HD), np.float16)
    vones = np.ones((P, TT, HPG), np.float16)

    in_maps = []
    for core in range(NCORES):
        b, g = core // G, core % G
        sl = slice(g * CL, (g + 1) * CL)
        idx = idxs[b]
        pad = np.zeros(NQ, np.int64)
        pad[:len(idx)] = idx
        if len(idx) < NQ:
            pad[len(idx):] = idx[0] if len(idx) else 0
        in_maps.append({
            "qxT": np.ascontiguousarray(q_x[b][pad].T.astype(np.float16)),
            "kvxT": np.ascontiguousarray(kv_x[b].T.astype(np.float16)),
            "wq": np.ascontiguousarray(Wq[sl].T.astype(np.float16)),
            "wk": np.ascontiguousarray(Wkv[sl].T.astype(np.float16)),
            "wv": np.ascontiguousarray(
                Wkv[C + g * CL:C + (g + 1) * CL].T.astype(np.float16)),
            "wp": np.ascontiguousarray(Wp[:, sl].T.astype(np.float16)),
            "colsel": colsel,
            "bcast": bcast,
            "vones": vones,
            "ones16": ones16,
        })
    return in_maps


_NC_CACHE = {}


def get_nc(NQ):
    if NQ not in _NC_CACHE:
        _NC_CACHE[NQ] = build_bass(NQ)
    return _NC_CACHE[NQ]


def prepare(q_x, kv_x, attn_mask, Wq, Wkv, Wp):
    mask = np.asarray(attn_mask).astype(bool)
    idxs = [np.flatnonzero(mask[b]) for b in range(B)]
    numax = max(1, max(len(i) for i in idxs))
    NQ = ((numax + 31) // 32) * 32
    nc = get_nc(NQ)
    in_maps = make_in_maps(q_x, kv_x, mask, Wq, Wkv, Wp, NQ, idxs)
    return nc, in_maps, idxs


def kernel(q_x, kv_x, attn_mask, Wq, Wkv, qn_w, qn_b, kn_w, kn_b, Wp, bp,
           _profile=None):
    q_x = np.asarray(q_x, np.float32)
    kv_x = np.asarray(kv_x, np.float32)
    Wq = np.asarray(Wq, np.float32)
    Wkv = np.asarray(Wkv, np.float32)
    Wp = np.asarray(Wp, np.float32)
    bp = np.asarray(bp, np.float32)
    if not (np.all(np.asarray(qn_w) == 1) and np.all(np.asarray(qn_b) == 0)
            and np.all(np.asarray(kn_w) == 1) and np.all(np.asarray(kn_b) == 0)):
        raise NotImplementedError("kernel specialized to identity q/k norms")

    nc, in_maps, idxs = prepare(q_x, kv_x, attn_mask, Wq, Wkv, Wp)
    res = bass_utils.run_bass_kernel_spmd(
        nc, in_maps, core_ids=list(range(NCORES)))
    if _profile is not None:
        _profile.append(res)

    # masked-query rows: softmax over an all -1e9 row is uniform, so the
    # output is exactly mean_k(v) @ Wp.T + bp -- pure host math.
    vmean = kv_x.mean(axis=1) @ Wkv[C:].T          # [B, C]
    ymask = vmean @ Wp.T + bp                      # [B, C]
    out = np.empty((B, N, C), np.float32)
    for b in range(B):
        acc = (res.results[G * b]["outT"].astype(np.float32)
               + res.results[G * b + 1]["outT"].astype(np.float32))
        out[b] = ymask[b]
        nb = len(idxs[b])
        out[b, idxs[b]] = acc.T[:nb] + bp
    return out
